# revision 1
# baseline (speedup 1.0000x reference)
"""Trainium2 Bass kernel for nn_GAT_77953656422757 (GATConv x4 + TopKPool x2).

Sharding: graph-level data parallel, 4 graphs per NeuronCore x 8 cores.
Pipeline: K0 (BN stats) -> K1 (BN+GAT1+GAT2+score1) -> host top-k ->
K2 (pool-scale+GAT3+GAT4+score2) -> host top-k -> K3 (pool+mean+log_softmax).
Attention uses the exact factorization exp(leaky_relu(a+b)) = max(e^a e^b, e^{a/5} e^{b/5})
so per-edge work is two multiplies and a max (custom fused DVE op), with
per-node exp tables and an all-zero sentinel row for padding.
Edge gathers run through the Q7 SWDGE dma_gather (256B f32 rows, <=1024 idx/call).
"""
import sys
sys.path.insert(0, "/opt/trn_rl_repo")
import numpy as np

"""Host-side preprocessing + numpy model of the kernel's math decomposition.

Validates the factorized attention (exp(leaky(z)) = max(u*v, u2*v2)),
sentinel padding, skipped max-subtraction, degree-sorted chunk layout,
and the 4-launch split against the jax reference.
"""
import numpy as np

B = 32
NPER = 2048
DEG = 16
NT = B * NPER
NF = 128
NCLS = 10
NCORES = 8
GPC = B // NCORES          # graphs per core = 4
NPCORE = GPC * NPER        # nodes per core = 8192
NCHUNK = NPCORE // 128     # 64 chunks per core
JCHUNK = NPER // 128       # 16 rank-slices per graph

SENT = NPCORE              # sentinel row index (per-core table has NPCORE+1 rows)


def build_core_graphs(edge_index):
    """Split the global edge list into per-core local edge lists (with self-loops)."""
    src_g = np.asarray(edge_index[0])
    dst_g = np.asarray(edge_index[1])
    E_per_graph = NPER * DEG
    cores = []
    for c in range(NCORES):
        base_node = c * NPCORE
        e0 = c * GPC * E_per_graph
        e1 = (c + 1) * GPC * E_per_graph
        src = src_g[e0:e1] - base_node
        dst = dst_g[e0:e1] - base_node
        sl = np.arange(NPCORE, dtype=np.int32)
        src = np.concatenate([src, sl]).astype(np.int64)
        dst = np.concatenate([dst, sl]).astype(np.int64)
        cores.append((src, dst))
    return cores


def degree_sort_perms(cores):
    """Per-core permutation: within each graph, nodes sorted by in-degree desc.
    perm[new_local_id] = old_local_id.   Returns perms and per-core in-degrees."""
    perms = []
    for (src, dst) in cores:
        deg = np.bincount(dst, minlength=NPCORE)
        perm = np.empty(NPCORE, dtype=np.int64)
        for g in range(GPC):
            lo, hi = g * NPER, (g + 1) * NPER
            order = np.argsort(-deg[lo:hi], kind="stable")
            perm[lo:hi] = lo + order
        perms.append(perm)
    return perms


def chunk_widths(cores, perms):
    """K_j for j in [0, JCHUNK): max (over all cores+graphs) in-degree at
    rank-slice j, so the compiled program is identical across cores."""
    K = np.zeros(JCHUNK, dtype=np.int64)
    for (src, dst), perm in zip(cores, perms):
        deg = np.bincount(dst, minlength=NPCORE)
        deg_sorted = deg[perm]  # new-id order, per-graph desc
        for g in range(GPC):
            d = deg_sorted[g * NPER:(g + 1) * NPER]
            for j in range(JCHUNK):
                K[j] = max(K[j], d[j * 128:(j + 1) * 128].max())
    # pad to even for alignment friendliness
    K = ((K + 1) // 2) * 2
    return K


def gather_groups(K):
    """Static per-chunk gather splitting: list per chunk of (slot_off, r) with
    r = slots per dma_gather call (<=8 so num_idxs = 128*r <= 1024)."""
    groups = []
    for c in range(NCHUNK):
        k = int(K[c % JCHUNK])
        g, s = [], 0
        while s < k:
            r = min(8, k - s)
            g.append((s, r))
            s += r
        groups.append(g)
    return groups


def build_idx_tables(core, perm, K):
    """Per-core slot tables.
    Returns (idx_cm int32 [128, S_tot] dst-major (for numpy model),
             idx16 int16 [128, W_tot] wrapped-16/replicated (for dma_gather),
             offs, goffs)."""
    src, dst = core
    inv = np.empty(NPCORE, dtype=np.int64)
    inv[perm] = np.arange(NPCORE)
    src_n = inv[src]
    dst_n = inv[dst]
    order = np.argsort(dst_n, kind="stable")
    src_s = src_n[order]
    deg = np.bincount(dst_n, minlength=NPCORE)
    starts = np.concatenate([[0], np.cumsum(deg)])
    offs = []
    S_tot = int(sum(K[c % JCHUNK] for c in range(NCHUNK)))
    idx_cm = np.full((128, S_tot), SENT, dtype=np.int32)
    off = 0
    for c in range(NCHUNK):
        j = c % JCHUNK
        k = int(K[j])
        offs.append(off)
        for p in range(128):
            d = 128 * c + p
            s, e = starts[d], starts[d + 1]
            assert e - s <= k, (c, p, e - s, k)
            idx_cm[p, off:off + (e - s)] = src_s[s:e]
        off += k
    # wrapped int16 layout for dma_gather, slot-major per gather group
    groups = gather_groups(K)
    W_tot = sum(8 * r for g in groups for (_, r) in g)
    idx16 = np.zeros((128, W_tot), np.int16)
    goffs = []
    w = 0
    for c in range(NCHUNK):
        go = []
        for (s0, r) in groups[c]:
            ni = 128 * r
            lin = np.empty(ni, np.int64)
            for k in range(r):
                lin[k * 128:(k + 1) * 128] = idx_cm[:, offs[c] + s0 + k]
            blk = np.tile(lin.reshape(ni // 16, 16).T, (8, 1)).astype(np.int16)
            idx16[:, w:w + ni // 16] = blk
            go.append((w, s0, r))
            w += ni // 16
        goffs.append(go)
    return idx_cm, idx16, offs, goffs


def leaky(x, a=0.2):
    return np.where(x > 0, x, a * x)


def np_gat_layer(x, idx_cm, offs, K, W, a_s, a_d, b, H, C, nmask):
    """Numpy model of the device layer: x [NPCORE, F] (new-id order),
    returns out [NPCORE, H*C].  nmask f32 [NPCORE]."""
    h = x @ W                                   # [N, H*C]
    asf = np.zeros((H * C, H), np.float32)
    adf = np.zeros((H * C, H), np.float32)
    for hh in range(H):
        asf[hh * C:(hh + 1) * C, hh] = a_s[hh]
        adf[hh * C:(hh + 1) * C, hh] = a_d[hh]
    als = h @ asf                               # [N, H]
    ald = h @ adf
    u = np.exp(als) * nmask[:, None]
    u2 = np.exp(0.2 * als) * nmask[:, None]
    v = np.exp(ald) * nmask[:, None]
    v2 = np.exp(0.2 * ald) * nmask[:, None]
    # sentinel row
    hT = np.concatenate([h, np.zeros((1, H * C), np.float32)], axis=0)
    uT = np.concatenate([u, np.zeros((1, H), np.float32)], axis=0)
    u2T = np.concatenate([u2, np.zeros((1, H), np.float32)], axis=0)
    out = np.zeros((NPCORE, H * C), np.float32)
    for c in range(NCHUNK):
        j = c % JCHUNK
        k = int(K[j])
        off = offs[c]
        idx = idx_cm[:, off:off + k]            # [128, k]
        g_h = hT[idx]                           # [128, k, H*C]
        g_u = uT[idx]                           # [128, k, H]
        g_u2 = u2T[idx]
        vv = v[c * 128:(c + 1) * 128]           # [128, H]
        vv2 = v2[c * 128:(c + 1) * 128]
        ex = np.maximum(g_u * vv[:, None, :], g_u2 * vv2[:, None, :])  # [128,k,H]
        den = ex.sum(axis=1)                    # [128, H]
        rden = 1.0 / np.maximum(den, 1e-16)
        exh = np.repeat(ex, C, axis=2)          # [128, k, H*C]
        pre = (exh * g_h).sum(axis=1)           # [128, H*C]
        out[c * 128:(c + 1) * 128] = pre * np.repeat(rden, C, axis=1)
    return out + b


def pack_asf(a, H, C):
    """block-diagonal [H*C, H] from a [H, C]."""
    m = np.zeros((H * C, H), np.float32)
    for h in range(H):
        m[h * C:(h + 1) * C, h] = a[h]
    return m


def prep_static(inputs):
    """All input-independent-of-device-results preprocessing, per core."""
    x = np.asarray(inputs["x"], np.float32)
    edge_index = np.asarray(inputs["edge_index"])
    cores = build_core_graphs(edge_index)
    perms = degree_sort_perms(cores)
    K = chunk_widths(cores, perms)
    percore = []
    for c in range(NCORES):
        idx_cm, idx16, offs, goffs = build_idx_tables(cores[c], perms[c], K)
        xc = x[c * NPCORE:(c + 1) * NPCORE][perms[c]]
        xT = np.ascontiguousarray(xc.T)
        percore.append(dict(idx_cm=idx_cm, idx16=idx16, offs=offs, goffs=goffs,
                            xT=xT, perm=perms[c]))
    packs = dict(
        asf1=pack_asf(np.asarray(inputs["as1"]), 3, 16), adf1=pack_asf(np.asarray(inputs["ad1"]), 3, 16),
        asf2=pack_asf(np.asarray(inputs["as2"]), 3, 16), adf2=pack_asf(np.asarray(inputs["ad2"]), 3, 16),
        asf3=pack_asf(np.asarray(inputs["as3"]), 2, 16), adf3=pack_asf(np.asarray(inputs["ad3"]), 2, 16),
        asf4=pack_asf(np.asarray(inputs["as4"]), 1, 10), adf4=pack_asf(np.asarray(inputs["ad4"]), 1, 10),
    )
    return dict(cores=cores, perms=perms, K=K, percore=percore, packs=packs)


def host_topk(score_cm_list, perms, nm_prev_cm_list, k):
    """score_cm [128, 64] per core -> nm_cm [128, 64] per core, exact reference
    tie semantics (original-id order)."""
    out = []
    for c in range(NCORES):
        score = score_cm_list[c].T.reshape(-1)  # new-id order
        nm_prev = nm_prev_cm_list[c].T.reshape(-1) if nm_prev_cm_list else np.ones(NPCORE, np.float32)
        perm = perms[c]
        nm = np.zeros(NPCORE, np.float32)
        for g in range(GPC):
            sl = slice(g * NPER, (g + 1) * NPER)
            s_orig = np.empty(NPER, np.float32)
            so = score[sl].copy()
            so[nm_prev[sl] == 0] = -1e9
            s_orig[perm[sl] - g * NPER] = so
            keep = np.argsort(-s_orig, kind="stable")[:k]
            km = np.zeros(NPER, np.float32)
            km[keep] = 1.0
            nm[sl] = km[perm[sl] - g * NPER] * nm_prev[sl]
        out.append(np.ascontiguousarray(nm.reshape(NCHUNK, 128).T))
    return out


def np_full_model(inputs):
    """Full numpy model in the exact decomposition the device will use."""
    x = np.asarray(inputs["x"], np.float32)
    edge_index = np.asarray(inputs["edge_index"])
    gamma = np.asarray(inputs["gamma"]); beta = np.asarray(inputs["beta"])
    cores = build_core_graphs(edge_index)
    perms = degree_sort_perms(cores)
    K = chunk_widths(cores, perms)

    # global BN stats from per-core partials
    s0 = np.zeros(NF, np.float64); s1 = np.zeros(NF, np.float64)
    for c in range(NCORES):
        xc = x[c * NPCORE:(c + 1) * NPCORE].astype(np.float32)
        s0 += xc.sum(0); s1 += (xc * xc).sum(0)
    mu = (s0 / NT).astype(np.float32)
    var = (s1 / NT).astype(np.float32) - mu * mu
    A = gamma / np.sqrt(var + 1e-5)
    Bb = beta - mu * A

    outs = []
    for c in range(NCORES):
        perm = perms[c]
        idx_cm, offs = build_idx_tables(cores[c], perm, K)
        xc = x[c * NPCORE:(c + 1) * NPCORE][perm]
        xb = xc * A + Bb
        ones = np.ones(NPCORE, np.float32)
        x1 = np_gat_layer(xb, idx_cm, offs, K, inputs["W1"], inputs["as1"], inputs["ad1"], inputs["b1"], 3, 16, ones)
        x2 = np_gat_layer(x1, idx_cm, offs, K, inputs["W2"], inputs["as2"], inputs["ad2"], inputs["b2"], 3, 16, ones)
        # pool1: host top-k on scores (computed on device in reality)
        p1 = np.asarray(inputs["p1"])
        score1 = (x2 @ p1) / np.linalg.norm(p1)
        nm1 = np.zeros(NPCORE, np.float32)
        for g in range(GPC):
            sl = slice(g * NPER, (g + 1) * NPER)
            # unpermute to original order for exact tie semantics
            s_orig = np.full(NPER, -1e9, np.float32)
            s_orig[perm[sl] - g * NPER] = score1[sl]
            keep = np.argsort(-s_orig, kind="stable")[:NPER // 2]
            keep_mask_orig = np.zeros(NPER, np.float32)
            keep_mask_orig[keep] = 1.0
            nm1[sl] = keep_mask_orig[perm[sl] - g * NPER]
        x3 = x2 * (np.tanh(score1) * nm1)[:, None]
        x3l = np_gat_layer(x3, idx_cm, offs, K, inputs["W3"], inputs["as3"], inputs["ad3"], inputs["b3"], 2, 16, nm1)
        x4l = np_gat_layer(x3l, idx_cm, offs, K, inputs["W4"], inputs["as4"], inputs["ad4"], inputs["b4"], 1, NCLS, nm1)
        p2 = np.asarray(inputs["p2"])
        score2 = (x4l @ p2) / np.linalg.norm(p2)
        nm2 = np.zeros(NPCORE, np.float32)
        for g in range(GPC):
            sl = slice(g * NPER, (g + 1) * NPER)
            s_orig = np.full(NPER, -1e9, np.float32)
            so = score2[sl].copy()
            so[nm1[sl] == 0] = -1e9
            s_orig[perm[sl] - g * NPER] = so
            keep = np.argsort(-s_orig, kind="stable")[:NPER // 4]
            keep_mask_orig = np.zeros(NPER, np.float32)
            keep_mask_orig[keep] = 1.0
            nm2[sl] = keep_mask_orig[perm[sl] - g * NPER] * nm1[sl]
        x5 = x4l * (np.tanh(score2) * nm2)[:, None]
        for g in range(GPC):
            sl = slice(g * NPER, (g + 1) * NPER)
            pooled = x5[sl].sum(0) / (NPER // 4)
            outs.append(pooled)
    pooled = np.stack(outs)  # [32, 10]
    m = pooled.max(axis=1, keepdims=True)
    lse = np.log(np.exp(pooled - m).sum(axis=1, keepdims=True)) + m
    return pooled - lse



# ===== custom DVE op =====
from concourse.dve_ops import OPS, DveOp, _SUB_OPCODE_FOR_NAME, _CUSTOM_DVE_ROW_BASE
from concourse.dve_spec import Spec, Src0, Src1, C0, C1, Zero, maxx, lower, _has_src1
from concourse.dve_uop import DveOpSpec
from concourse.dve_table_gen import dve_ver_for
from operator import add



def _ref_ex_den(in0, in1, s0, s1, imm2):
    b = np.maximum(in0.astype(np.float32) * s0, in1.astype(np.float32) * s1).astype(
        np.float32
    )
    return b, b.reshape(b.shape[0], -1).sum(axis=-1, keepdims=True).astype(np.float32)


def register_ex_den():
    name = "GAT_EX_DEN_ANT"
    for op in OPS:
        if op.name == name:
            return op
    spec = Spec(
        body=maxx(Src0 * C0, Src1 * C1),
        accum=add,
        accum_init=Zero,
        reference=_ref_ex_den,
    )
    op = DveOp(name, spec, subdim=False, uops_sha={})
    OPS.append(op)
    _SUB_OPCODE_FOR_NAME[name] = _CUSTOM_DVE_ROW_BASE + len(OPS) - 1
    from concourse.dve_ops import CUSTOM_DVE_SPECS
    CUSTOM_DVE_SPECS[name] = spec
    for ver in ("v3",):
        tmp = DveOpSpec(
            name=name,
            opcode=_SUB_OPCODE_FOR_NAME[name],
            uops=lower(spec, ver=ver),
            rd1_en=_has_src1(spec),
        )
        op.uops_sha[ver] = tmp.sha(ver)
    return op


EX_DEN = register_ex_den()


# ===== kernels =====

import concourse.bass as bass
import concourse.bacc as bacc
import concourse.mybir as mybir
import concourse.tile as tile
from concourse.bass import IndirectOffsetOnAxis
from concourse import library_config
from concourse.tile_rust import add_dep_helper
from concourse.masks import make_identity
from concourse.dve_ops import TENSOR_TENSOR_REDUCE



F32 = mybir.dt.float32
BF16 = mybir.dt.bfloat16
I32 = mybir.dt.int32
I16 = mybir.dt.int16
AF = mybir.ActivationFunctionType
ALU = mybir.AluOpType
AX = mybir.AxisListType

NPCORE = 8192
NCHUNK = 64
JCHUNK = 16
GPC = 4
NPER = 2048
NT = 65536

# layer configs: table rows are 64 f32 (256B, dma_gather elem); UC = col of u[0]
ROWW = 64
LCFG = {
    1: dict(Cin=128, Cout=48, H=3, UC=48),
    2: dict(Cin=48, Cout=48, H=3, UC=48),
    3: dict(Cin=48, Cout=32, H=2, UC=32),
    4: dict(Cin=32, Cout=10, H=1, UC=10),
}


def new_nc():
    return bacc.Bacc("TRN2", target_bir_lowering=False, debug=False, num_devices=8)


def chunk_off(K):
    """column offsets of chunk c in the idx table; K = per-rank-slice widths."""
    offs, off = [], 0
    for c in range(NCHUNK):
        offs.append(off)
        off += int(K[c % JCHUNK])
    return offs, off


def gather_groups(K):
    groups = []
    for c in range(NCHUNK):
        k = int(K[c % JCHUNK])
        g, s = [], 0
        while s < k:
            r = min(8, k - s)
            g.append((s, r))
            s += r
        groups.append(g)
    return groups


def goffs_of(K):
    """static gather-group column offsets in the idx16 table (shared by cores)."""
    groups = gather_groups(K)
    goffs, w = [], 0
    for c in range(NCHUNK):
        go = []
        for (s0, r) in groups[c]:
            go.append((w, s0, r))
            w += 8 * r
        goffs.append(go)
    return goffs, w


# ---------------------------------------------------------------- K0: BN stats
def build_k0():
    nc = new_nc()
    xT = nc.dram_tensor("xT", [128, NPCORE], F32, kind="ExternalInput")
    stats = nc.dram_tensor("stats", [128, 2], F32, kind="ExternalOutput")
    with tile.TileContext(nc) as tc:
        with tc.tile_pool(name="p", bufs=1) as pool:
            xt = pool.tile([128, NPCORE], F32)
            nc.sync.dma_start(xt[:], xT.ap())
            sq = pool.tile([128, NPCORE], F32)
            nc.scalar.activation(sq[:], xt[:], AF.Square)
            st = pool.tile([128, 2], F32)
            nc.vector.tensor_reduce(st[:, 0:1], xt[:], axis=AX.X, op=ALU.add)
            nc.vector.tensor_reduce(st[:, 1:2], sq[:], axis=AX.X, op=ALU.add)
            nc.sync.dma_start(stats.ap(), st[:])
    nc.compile()
    return nc


# ------------------------------------------------------------ shared emitters
class LayerCtx:
    def __init__(self, nc, tc, pools, ident):
        self.nc, self.tc, self.pools, self.ident = nc, tc, pools, ident


def emit_wfull(nc, pools, W_ap, WT_ap, asf_ap, adf_ap, Cin, Cout, H, name):
    """Build Wfull [Cin, Cout+2H] sbuf tile = [W | W@asf | W@adf]."""
    wf = pools["const"].tile([Cin, Cout + 2 * H], F32, name=f"wf{name}")
    nc.sync.dma_start(wf[:, 0:Cout], W_ap)
    ps = pools["psum"].tile([Cin, 2 * H], F32, space="PSUM", name=f"wps{name}", tag="pp")
    nc.tensor.matmul(ps[:, 0:H], lhsT=WT_ap, rhs=asf_ap, start=True, stop=True)
    nc.tensor.matmul(ps[:, H:2 * H], lhsT=WT_ap, rhs=adf_ap, start=True, stop=True)
    nc.vector.tensor_copy(wf[:, Cout:Cout + 2 * H], ps[:])
    return wf


def emit_produce(ctx, li, c, xT_chunk_ap, wf, T_dram, vv, hbias=None, scale_col=None,
                 nm_col=None, trow_writes=None):
    """Produce table rows for chunk c of layer li and write to T_dram."""
    nc, pools = ctx.nc, ctx.pools
    cfg = LCFG[li]
    Cout, H, UC = cfg["Cout"], cfg["H"], cfg["UC"]
    WF = Cout + 2 * H
    ps = pools["psum"].tile([128, WF], F32, space="PSUM", name=f"pps{li}", tag="pp")
    nc.tensor.matmul(ps[:], lhsT=xT_chunk_ap, rhs=wf[:], start=True, stop=True)
    srow = pools["srow"].tile([128, WF], F32, name=f"srow{li}", tag="srow")
    if hbias is not None:
        nc.vector.tensor_tensor(srow[:], ps[:], hbias, op=ALU.add)
    elif scale_col is not None:
        nc.vector.tensor_scalar_mul(srow[:], ps[:], scale_col)
    else:
        nc.vector.tensor_copy(srow[:], ps[:])
    trow = pools["trow"].tile([128, ROWW], F32, name=f"trow{li}", tag="trow")
    nc.vector.memset(trow[:, UC + 2 * H:ROWW], 0)
    nc.vector.tensor_copy(trow[:, 0:Cout], srow[:, 0:Cout])
    # u = exp(als), u2 = exp(0.2 als)
    nc.scalar.activation(trow[:, UC:UC + H], srow[:, Cout:Cout + H], AF.Exp)
    nc.scalar.activation(trow[:, UC + H:UC + 2 * H], srow[:, Cout:Cout + H], AF.Exp, scale=0.2)
    # v = exp(ald), v2 = exp(0.2 ald) -> resident vv [128, NCHUNK, 2H]
    nc.scalar.activation(vv[:, c, 0:H], srow[:, Cout + H:Cout + 2 * H], AF.Exp)
    nc.scalar.activation(vv[:, c, H:2 * H], srow[:, Cout + H:Cout + 2 * H], AF.Exp, scale=0.2)
    if nm_col is not None:
        nc.vector.tensor_scalar_mul(trow[:, UC:UC + 2 * H], trow[:, UC:UC + 2 * H], nm_col)
        nc.vector.tensor_scalar_mul(vv[:, c, 0:2 * H], vv[:, c, 0:2 * H], nm_col)
    w = nc.sync.dma_start(T_dram.ap()[c * 128:(c + 1) * 128, :], trow[:])
    if trow_writes is not None:
        trow_writes.append(w)


def emit_attention(ctx, li, c, K, groups, idx_tile, T_dram, vv, brow, gather_dep=None):
    """Attention for chunk c of layer li; returns rows tile [128, Cout] f32.
    groups = list of (idx16_col_off, slot_off, r) dma_gather calls."""
    nc, pools = ctx.nc, ctx.pools
    cfg = LCFG[li]
    Cout, H, UC = cfg["Cout"], cfg["H"], cfg["UC"]
    RB = ROWW
    CperH = Cout // H
    gbuf = pools["gbuf"].tile([128, K * RB], F32, name=f"gbuf{li}", tag="gbuf")
    for (w0, s0, r) in groups:
        gi = nc.gpsimd.dma_gather(
            gbuf[:, s0 * RB:(s0 + r) * RB].rearrange("p (k r) -> p k r", r=RB),
            T_dram.ap(), idx_tile[:, w0:w0 + 8 * r], 128 * r, 128 * r, RB)
        for dep in (gather_dep or []):
            add_dep_helper(gi.ins, dep.ins, True, "dep before gather")
    g32 = gbuf[:].rearrange("p (k r) -> p k r", r=RB)
    ex = pools["ex"].tile([128, H * K], F32, name=f"ex{li}", tag="ex")
    den = pools["den"].tile([128, H], F32, name=f"den{li}", tag="den")
    for h in range(H):
        nc.vector._custom_dve(
            EX_DEN, out=ex[:, h * K:(h + 1) * K],
            in0=g32[:, :, UC + h], in1=g32[:, :, UC + H + h],
            s0=vv[:, c, h:h + 1], s1=vv[:, c, H + h:H + h + 1],
            accum_out=den[:, h:h + 1])
    rden = pools["den"].tile([128, H], F32, name=f"rden{li}", tag="rden")
    nc.vector.tensor_scalar_max(rden[:], den[:], 1e-16)
    nc.vector.reciprocal(rden[:], rden[:])
    # big multiply: tmp[p,k,f] = h_g[p,k,f] * ex[p,h(f),k]
    tmp = pools["tmp"].tile([128, K * Cout], F32, name=f"tmp{li}", tag="tmp")
    h_g = g32[:, :, 0:Cout]
    h_g4 = h_g.rearrange("p k (h cc) -> p k h cc", h=H)
    ex4 = ex[:].rearrange("p (h k) -> p k h", h=H).unsqueeze(3).broadcast_to(
        [128, K, H, CperH])
    tmp4 = tmp[:].rearrange("p (k h cc) -> p k h cc", h=H, cc=CperH)
    nc.vector.tensor_tensor(tmp4, h_g4, ex4, op=ALU.mult)
    pre = pools["pre"].tile([128, Cout], F32, name=f"pre{li}", tag="pre")
    tmp_r = tmp[:].rearrange("p (k f) -> p f k", f=Cout)
    nc.vector.tensor_reduce(pre[:], tmp_r, axis=AX.X, op=ALU.add)
    rows = pools["rows"].tile([128, Cout], F32, name=f"rows{li}", tag="rows")
    rden3 = rden[:].unsqueeze(2).broadcast_to([128, H, CperH])
    nc.vector.tensor_tensor(rows[:].rearrange("p (h cc) -> p h cc", h=H), pre[:].rearrange("p (h cc) -> p h cc", h=H), rden3, op=ALU.mult)
    nc.vector.tensor_tensor(rows[:], rows[:], brow, op=ALU.add)
    return rows


def emit_transpose(ctx, rows, Cout, dst_ap):
    """rows [128, Cout] -> dst_ap [Cout, 128] via PE transpose."""
    nc, pools = ctx.nc, ctx.pools
    pt = pools["psum"].tile([Cout, 128], F32, space="PSUM", name="ptr", tag="ptr")
    nc.tensor.transpose(out=pt[:], in_=rows[:], identity=ctx.ident[:])
    nc.vector.tensor_copy(dst_ap, pt[:])


def emit_rnorm(nc, pools, p_tile, C):
    """rnorm [128,1] = 1/||p|| from broadcast p_tile [128, C]."""
    sq = pools["const"].tile([128, C], F32, name=f"pn{C}")
    nc.vector.tensor_tensor(sq[:], p_tile[:], p_tile[:], op=ALU.mult)
    nrm = pools["const"].tile([128, 2], F32, name=f"nrm{C}")
    nc.vector.tensor_reduce(nrm[:, 0:1], sq[:], axis=AX.X, op=ALU.add)
    nc.scalar.activation(nrm[:, 1:2], nrm[:, 0:1], AF.Sqrt)
    nc.vector.reciprocal(nrm[:, 0:1], nrm[:, 1:2])
    return nrm


def emit_score(ctx, rows, Cout, p_tile, rnorm, score_out_col):
    nc = ctx.nc
    sctmp = ctx.pools["tmp"].tile([128, Cout], F32, name="sctmp", tag="sctmp")
    nc.vector._custom_dve(
        TENSOR_TTR, out=sctmp[:],
        in0=rows[:], in1=p_tile[:],
        s0=0.0, s1=rnorm[:, 0:1],
        accum_out=score_out_col)


TENSOR_TTR = TENSOR_TENSOR_REDUCE


# ------------------------------------------------------------------- K1
def build_k1(Kw, reps=1):
    """BN + L1 + L2 + score1.  Kw = per-rank-slice widths [16]."""
    goffs, W_tot = goffs_of(Kw)
    nc = new_nc()
    xT_d = nc.dram_tensor("xT", [128, NPCORE], F32, kind="ExternalInput")
    stats_d = nc.dram_tensor("stats", [128, 2], F32, kind="ExternalInput")
    gamma_d = nc.dram_tensor("gamma", [128, 1], F32, kind="ExternalInput")
    beta_d = nc.dram_tensor("beta", [128, 1], F32, kind="ExternalInput")
    W1_d = nc.dram_tensor("W1", [128, 48], F32, kind="ExternalInput")
    W1T_d = nc.dram_tensor("W1T", [48, 128], F32, kind="ExternalInput")
    W2_d = nc.dram_tensor("W2", [48, 48], F32, kind="ExternalInput")
    W2T_d = nc.dram_tensor("W2T", [48, 48], F32, kind="ExternalInput")
    asf1_d = nc.dram_tensor("asf1", [48, 3], F32, kind="ExternalInput")
    adf1_d = nc.dram_tensor("adf1", [48, 3], F32, kind="ExternalInput")
    asf2_d = nc.dram_tensor("asf2", [48, 3], F32, kind="ExternalInput")
    adf2_d = nc.dram_tensor("adf2", [48, 3], F32, kind="ExternalInput")
    b1_d = nc.dram_tensor("b1", [1, 48], F32, kind="ExternalInput")
    b2_d = nc.dram_tensor("b2", [1, 48], F32, kind="ExternalInput")
    p1_d = nc.dram_tensor("p1", [1, 48], F32, kind="ExternalInput")
    idx_d = nc.dram_tensor("idx", [128, W_tot], I16, kind="ExternalInput")
    T1_d = nc.dram_tensor("T1", [NPCORE + 1, ROWW], F32, kind="Internal")
    T2_d = nc.dram_tensor("T2", [NPCORE + 1, ROWW], F32, kind="Internal")
    x3T_d = nc.dram_tensor("x3T", [48, NPCORE], F32, kind="ExternalOutput")
    sc1_d = nc.dram_tensor("score1", [128, NCHUNK], F32, kind="ExternalOutput")

    with tile.TileContext(nc) as tc:
        with (
            tc.tile_pool(name="const", bufs=1) as constp,
            tc.tile_pool(name="psum", bufs=4, space="PSUM") as psump,
            tc.tile_pool(name="xin", bufs=4) as xinp,
            tc.tile_pool(name="srow", bufs=4) as srowp,
            tc.tile_pool(name="trow", bufs=4) as trowp,
            tc.tile_pool(name="gbuf", bufs=6) as gbufp,
            tc.tile_pool(name="ex", bufs=8) as exp_,
            tc.tile_pool(name="den", bufs=8) as denp,
            tc.tile_pool(name="tmp", bufs=4) as tmpp,
            tc.tile_pool(name="pre", bufs=4) as prep,
            tc.tile_pool(name="rows", bufs=4) as rowsp,
            tc.tile_pool(name="res", bufs=1) as resp,
        ):
            pools = dict(const=constp, psum=psump, xin=xinp, srow=srowp, trow=trowp,
                         gbuf=gbufp, ex=exp_, den=denp, tmp=tmpp, pre=prep, rows=rowsp)
            ident = constp.tile([128, 128], F32, name="ident")
            make_identity(nc, ident[:])
            ctx = LayerCtx(nc, tc, pools, ident)

            # resident tiles
            idx_t = resp.tile([128, W_tot], I16, name="idxt")
            nc.sync.dma_start(idx_t[:], idx_d.ap())
            lib = nc.gpsimd.load_library(library_config.mlp)
            x2T = resp.tile([48, NPCORE], F32, name="x2T")
            vv1 = resp.tile([128, NCHUNK, 6], F32, name="vv1")
            vv2_ = resp.tile([128, NCHUNK, 6], F32, name="vv2")
            zrow = constp.tile([1, ROWW], F32, name="zrow")
            nc.vector.memset(zrow[:], 0)
            gam = constp.tile([128, 1], F32, name="gam")
            nc.sync.dma_start(gam[:], gamma_d.ap())
            bet = constp.tile([128, 1], F32, name="bet")
            nc.sync.dma_start(bet[:], beta_d.ap())
            stats_t = constp.tile([128, 2], F32, name="stats")
            nc.sync.dma_start(stats_t[:], stats_d.ap())
            b1_t = constp.tile([128, 48], F32, name="b1")
            nc.sync.dma_start(b1_t[:], b1_d.ap().to_broadcast([128, 48]))
            b2_t = constp.tile([128, 48], F32, name="b2")
            nc.sync.dma_start(b2_t[:], b2_d.ap().to_broadcast([128, 48]))
            p1_t = constp.tile([128, 48], F32, name="p1")
            nc.sync.dma_start(p1_t[:], p1_d.ap().to_broadcast([128, 48]))
            asf1_t = constp.tile([48, 3], F32, name="asf1")
            nc.sync.dma_start(asf1_t[:], asf1_d.ap())
            adf1_t = constp.tile([48, 3], F32, name="adf1")
            nc.sync.dma_start(adf1_t[:], adf1_d.ap())
            asf2_t = constp.tile([48, 3], F32, name="asf2")
            nc.sync.dma_start(asf2_t[:], asf2_d.ap())
            adf2_t = constp.tile([48, 3], F32, name="adf2")
            nc.sync.dma_start(adf2_t[:], adf2_d.ap())
            w1t_t = constp.tile([48, 128], F32, name="w1t")
            nc.sync.dma_start(w1t_t[:], W1T_d.ap())
            w2t_t = constp.tile([48, 48], F32, name="w2t")
            nc.sync.dma_start(w2t_t[:], W2T_d.ap())

            def body(it=None):
                # sentinel rows
                s1w = nc.sync.dma_start(T1_d.ap()[NPCORE:NPCORE + 1, :], zrow[:])
                s2w = nc.sync.dma_start(T2_d.ap()[NPCORE:NPCORE + 1, :], zrow[:])
                # BN fold
                ab = constp.tile([128, 6], F32, name="ab")
                nc.vector.tensor_scalar_mul(ab[:, 0:2], stats_t[:], 1.0 / NT)  # mu, ex2
                nc.vector.tensor_tensor(ab[:, 2:3], ab[:, 0:1], ab[:, 0:1], op=ALU.mult)
                nc.vector.tensor_tensor(ab[:, 2:3], ab[:, 1:2], ab[:, 2:3], op=ALU.subtract)  # var
                nc.vector.tensor_scalar_add(ab[:, 2:3], ab[:, 2:3], 1e-5)
                nc.scalar.activation(ab[:, 3:4], ab[:, 2:3], AF.Sqrt)
                nc.vector.reciprocal(ab[:, 3:4], ab[:, 3:4])  # rs
                nc.vector.tensor_tensor(ab[:, 4:5], gam[:], ab[:, 3:4], op=ALU.mult)  # A
                nc.vector.tensor_tensor(ab[:, 5:6], ab[:, 0:1], ab[:, 4:5], op=ALU.mult)
                nc.vector.tensor_tensor(ab[:, 5:6], bet[:], ab[:, 5:6], op=ALU.subtract)  # B

                wf1 = emit_wfull(nc, pools, W1_d.ap(), w1t_t[:], asf1_t[:], adf1_t[:], 128, 48, 3, "1")
                nc.vector.tensor_scalar_mul(wf1[:], wf1[:], ab[:, 4:5])
                hb_ps = psump.tile([1, 54], F32, space="PSUM", name="hbps", tag="pp")
                nc.tensor.matmul(hb_ps[:], lhsT=ab[:, 5:6], rhs=wf1[:], start=True, stop=True)
                hb1r = constp.tile([1, 54], F32, name="hb1r")
                nc.vector.tensor_copy(hb1r[:], hb_ps[:])
                onesrow = constp.tile([1, 128], F32, name="onesrow")
                nc.vector.memset(onesrow[:], 1.0)
                hbb_ps = psump.tile([128, 54], F32, space="PSUM", name="hbbps", tag="pp")
                nc.tensor.matmul(hbb_ps[:], lhsT=onesrow[:], rhs=hb1r[:], start=True, stop=True)
                hb1 = constp.tile([128, 54], F32, name="hb1")
                nc.vector.tensor_copy(hb1[:], hbb_ps[:])
                wf2 = emit_wfull(nc, pools, W2_d.ap(), w2t_t[:], asf2_t[:], adf2_t[:], 48, 48, 3, "2")
                rn1 = emit_rnorm(nc, pools, p1_t[:], 48)

                # ---- L1 produce
                t1w = [s1w]
                for c in range(NCHUNK):
                    xt = xinp.tile([128, 128], F32, name="xt")
                    nc.sync.dma_start(xt[:], xT_d.ap()[:, c * 128:(c + 1) * 128])
                    emit_produce(ctx, 1, c, xt[:], wf1, T1_d, vv1[:], hbias=hb1[:],
                                 trow_writes=t1w)
                bar1 = nc.vector.memset(constp.tile([1, 1], F32, name="barr1")[:], 0)
                for w in t1w:
                    add_dep_helper(bar1.ins, w.ins, True, "T1 writes before gathers")
                # ---- L1 attention -> x2T
                for c in range(NCHUNK):
                    K = int(Kw[c % JCHUNK])
                    rows = emit_attention(ctx, 1, c, K, goffs[c], idx_t, T1_d, vv1[:],
                                          b1_t[:], gather_dep=[bar1, lib])
                    emit_transpose(ctx, rows, 48, x2T[:, c * 128:(c + 1) * 128])
                # ---- L2 produce
                t2w = [s2w]
                for c in range(NCHUNK):
                    emit_produce(ctx, 2, c, x2T[:, c * 128:(c + 1) * 128], wf2, T2_d,
                                 vv2_[:], trow_writes=t2w)
                bar2 = nc.vector.memset(constp.tile([1, 1], F32, name="barr2")[:], 0)
                for w in t2w:
                    add_dep_helper(bar2.ins, w.ins, True, "T2 writes before gathers")
                # ---- L2 attention -> x3T out + score1
                sc = resp.tile([128, NCHUNK], F32, name="sc")
                for c in range(NCHUNK):
                    K = int(Kw[c % JCHUNK])
                    rows = emit_attention(ctx, 2, c, K, goffs[c], idx_t, T2_d, vv2_[:],
                                          b2_t[:], gather_dep=[bar2, lib])
                    x3c = pools["pre"].tile([48, 128], F32, name="x3c", tag="x3c")
                    emit_transpose(ctx, rows, 48, x3c[:])
                    nc.sync.dma_start(x3T_d.ap()[:, c * 128:(c + 1) * 128], x3c[:])
                    emit_score(ctx, rows, 48, p1_t[:], rn1, sc[:, c:c + 1])
                nc.sync.dma_start(sc1_d.ap(), sc[:])

            if reps > 1:
                with tc.For_i(0, reps) as _:
                    body()
            else:
                body()
    nc.compile()
    return nc


# ------------------------------------------------------------------- K2
def build_k2(Kw, reps=1):
    """pool1-scale + L3 + L4 + score2."""
    goffs, W_tot = goffs_of(Kw)
    nc = new_nc()
    x3T_d = nc.dram_tensor("x3T", [48, NPCORE], F32, kind="ExternalInput")
    sc1_d = nc.dram_tensor("score1", [128, NCHUNK], F32, kind="ExternalInput")
    nm1_d = nc.dram_tensor("nm1", [128, NCHUNK], F32, kind="ExternalInput")
    W3_d = nc.dram_tensor("W3", [48, 32], F32, kind="ExternalInput")
    W3T_d = nc.dram_tensor("W3T", [32, 48], F32, kind="ExternalInput")
    W4_d = nc.dram_tensor("W4", [32, 10], F32, kind="ExternalInput")
    W4T_d = nc.dram_tensor("W4T", [10, 32], F32, kind="ExternalInput")
    asf3_d = nc.dram_tensor("asf3", [32, 2], F32, kind="ExternalInput")
    adf3_d = nc.dram_tensor("adf3", [32, 2], F32, kind="ExternalInput")
    asf4_d = nc.dram_tensor("asf4", [10, 1], F32, kind="ExternalInput")
    adf4_d = nc.dram_tensor("adf4", [10, 1], F32, kind="ExternalInput")
    b3_d = nc.dram_tensor("b3", [1, 32], F32, kind="ExternalInput")
    b4_d = nc.dram_tensor("b4", [1, 10], F32, kind="ExternalInput")
    p2_d = nc.dram_tensor("p2", [1, 10], F32, kind="ExternalInput")
    idx_d = nc.dram_tensor("idx", [128, W_tot], I16, kind="ExternalInput")
    T3_d = nc.dram_tensor("T3", [NPCORE + 1, ROWW], F32, kind="Internal")
    T4_d = nc.dram_tensor("T4", [NPCORE + 1, ROWW], F32, kind="Internal")
    x5_d = nc.dram_tensor("x5", [128, NCHUNK * 10], F32, kind="ExternalOutput")
    sc2_d = nc.dram_tensor("score2", [128, NCHUNK], F32, kind="ExternalOutput")

    with tile.TileContext(nc) as tc:
        with (
            tc.tile_pool(name="const", bufs=1) as constp,
            tc.tile_pool(name="psum", bufs=4, space="PSUM") as psump,
            tc.tile_pool(name="srow", bufs=4) as srowp,
            tc.tile_pool(name="trow", bufs=4) as trowp,
            tc.tile_pool(name="gbuf", bufs=4) as gbufp,
            tc.tile_pool(name="ex", bufs=8) as exp_,
            tc.tile_pool(name="den", bufs=8) as denp,
            tc.tile_pool(name="tmp", bufs=3) as tmpp,
            tc.tile_pool(name="pre", bufs=4) as prep,
            tc.tile_pool(name="rows", bufs=4) as rowsp,
            tc.tile_pool(name="res", bufs=1) as resp,
        ):
            pools = dict(const=constp, psum=psump, srow=srowp, trow=trowp,
                         gbuf=gbufp, ex=exp_, den=denp, tmp=tmpp, pre=prep, rows=rowsp)
            ident = constp.tile([128, 128], F32, name="ident")
            make_identity(nc, ident[:])
            ctx = LayerCtx(nc, tc, pools, ident)

            idx_t = resp.tile([128, W_tot], I16, name="idxt")
            nc.sync.dma_start(idx_t[:], idx_d.ap())
            lib = nc.gpsimd.load_library(library_config.mlp)
            x3T = resp.tile([48, NPCORE], F32, name="x3T")
            nc.sync.dma_start(x3T[:], x3T_d.ap())
            sc1_t = resp.tile([128, NCHUNK], F32, name="sc1")
            nc.sync.dma_start(sc1_t[:], sc1_d.ap())
            nm1_t = resp.tile([128, NCHUNK], F32, name="nm1")
            nc.sync.dma_start(nm1_t[:], nm1_d.ap())
            x4T = resp.tile([32, NPCORE], F32, name="x4T")
            vv3 = resp.tile([128, NCHUNK, 4], F32, name="vv3")
            vv4 = resp.tile([128, NCHUNK, 2], F32, name="vv4")
            zrow = constp.tile([1, ROWW], F32, name="zrow")
            nc.vector.memset(zrow[:], 0)
            b3_t = constp.tile([128, 32], F32, name="b3")
            nc.sync.dma_start(b3_t[:], b3_d.ap().to_broadcast([128, 32]))
            b4_t = constp.tile([128, 10], F32, name="b4")
            nc.sync.dma_start(b4_t[:], b4_d.ap().to_broadcast([128, 10]))
            p2_t = constp.tile([128, 10], F32, name="p2")
            nc.sync.dma_start(p2_t[:], p2_d.ap().to_broadcast([128, 10]))
            asf3_t = constp.tile([32, 2], F32, name="asf3")
            nc.sync.dma_start(asf3_t[:], asf3_d.ap())
            adf3_t = constp.tile([32, 2], F32, name="adf3")
            nc.sync.dma_start(adf3_t[:], adf3_d.ap())
            asf4_t = constp.tile([10, 1], F32, name="asf4")
            nc.sync.dma_start(asf4_t[:], asf4_d.ap())
            adf4_t = constp.tile([10, 1], F32, name="adf4")
            nc.sync.dma_start(adf4_t[:], adf4_d.ap())
            w3t_t = constp.tile([32, 48], F32, name="w3t")
            nc.sync.dma_start(w3t_t[:], W3T_d.ap())
            w4t_t = constp.tile([10, 32], F32, name="w4t")
            nc.sync.dma_start(w4t_t[:], W4T_d.ap())

            def body(it=None):
                s3w = nc.sync.dma_start(T3_d.ap()[NPCORE:NPCORE + 1, :], zrow[:])
                s4w = nc.sync.dma_start(T4_d.ap()[NPCORE:NPCORE + 1, :], zrow[:])
                wf3 = emit_wfull(nc, pools, W3_d.ap(), w3t_t[:], asf3_t[:], adf3_t[:], 48, 32, 2, "3")
                wf4 = emit_wfull(nc, pools, W4_d.ap(), w4t_t[:], asf4_t[:], adf4_t[:], 32, 10, 1, "4")
                rn2 = emit_rnorm(nc, pools, p2_t[:], 10)
                # s = tanh(score1) * nm1   [128, NCHUNK]
                s_t = resp.tile([128, NCHUNK], F32, name="s_t")
                nc.scalar.activation(s_t[:], sc1_t[:], AF.Tanh)
                nc.vector.tensor_tensor(s_t[:], s_t[:], nm1_t[:], op=ALU.mult)

                # ---- L3 produce (scale rows by s, mask u/v by nm1)
                t3w = [s3w]
                for c in range(NCHUNK):
                    emit_produce(ctx, 3, c, x3T[:, c * 128:(c + 1) * 128], wf3, T3_d,
                                 vv3[:], scale_col=s_t[:, c:c + 1],
                                 nm_col=nm1_t[:, c:c + 1], trow_writes=t3w)
                bar3 = nc.vector.memset(constp.tile([1, 1], F32, name="barr3")[:], 0)
                for w in t3w:
                    add_dep_helper(bar3.ins, w.ins, True, "T3 writes before gathers")
                for c in range(NCHUNK):
                    K = int(Kw[c % JCHUNK])
                    rows = emit_attention(ctx, 3, c, K, goffs[c], idx_t, T3_d, vv3[:],
                                          b3_t[:], gather_dep=[bar3, lib])
                    emit_transpose(ctx, rows, 32, x4T[:, c * 128:(c + 1) * 128])
                # ---- L4
                t4w = [s4w]
                for c in range(NCHUNK):
                    emit_produce(ctx, 4, c, x4T[:, c * 128:(c + 1) * 128], wf4, T4_d,
                                 vv4[:], nm_col=nm1_t[:, c:c + 1], trow_writes=t4w)
                bar4 = nc.vector.memset(constp.tile([1, 1], F32, name="barr4")[:], 0)
                for w in t4w:
                    add_dep_helper(bar4.ins, w.ins, True, "T4 writes before gathers")
                x5st = resp.tile([128, NCHUNK * 10], F32, name="x5st")
                sc2 = resp.tile([128, NCHUNK], F32, name="sc2")
                for c in range(NCHUNK):
                    K = int(Kw[c % JCHUNK])
                    rows = emit_attention(ctx, 4, c, K, goffs[c], idx_t, T4_d, vv4[:],
                                          b4_t[:], gather_dep=[bar4, lib])
                    nc.vector.tensor_copy(x5st[:, c * 10:(c + 1) * 10], rows[:])
                    emit_score(ctx, rows, 10, p2_t[:], rn2, sc2[:, c:c + 1])
                nc.sync.dma_start(x5_d.ap(), x5st[:])
                nc.sync.dma_start(sc2_d.ap(), sc2[:])

            if reps > 1:
                with tc.For_i(0, reps) as _:
                    body()
            else:
                body()
    nc.compile()
    return nc


# ------------------------------------------------------------------- K3
def build_k3():
    """pool2 apply + per-graph mean + log_softmax -> [GPC, 10]."""
    nc = new_nc()
    x5_d = nc.dram_tensor("x5", [128, NCHUNK * 10], F32, kind="ExternalInput")
    sc2_d = nc.dram_tensor("score2", [128, NCHUNK], F32, kind="ExternalInput")
    nm2_d = nc.dram_tensor("nm2", [128, NCHUNK], F32, kind="ExternalInput")
    out_d = nc.dram_tensor("out", [GPC, 10], F32, kind="ExternalOutput")
    CPG = NPER // 128  # chunks per graph = 16
    with tile.TileContext(nc) as tc:
        with (
            tc.tile_pool(name="p", bufs=1) as pool,
            tc.tile_pool(name="psum", bufs=4, space="PSUM") as psump,
        ):
            x5 = pool.tile([128, NCHUNK * 10], F32, name="x5")
            nc.sync.dma_start(x5[:], x5_d.ap())
            sc2 = pool.tile([128, NCHUNK], F32, name="sc2")
            nc.sync.dma_start(sc2[:], sc2_d.ap())
            nm2 = pool.tile([128, NCHUNK], F32, name="nm2")
            nc.sync.dma_start(nm2[:], nm2_d.ap())
            ident = pool.tile([128, 128], F32, name="ident")
            make_identity(nc, ident[:])
            s = pool.tile([128, NCHUNK], F32, name="s")
            nc.scalar.activation(s[:], sc2[:], AF.Tanh)
            nc.vector.tensor_tensor(s[:], s[:], nm2[:], op=ALU.mult)
            xn = pool.tile([128, NCHUNK * 10], F32, name="xn")
            x5_3 = x5[:].rearrange("p (c f) -> p c f", f=10)
            s3 = s[:].unsqueeze(2).broadcast_to([128, NCHUNK, 10])
            nc.vector.tensor_tensor(xn[:].rearrange("p (c f) -> p c f", f=10), x5_3, s3, op=ALU.mult)
            ones = pool.tile([128, 1], F32, name="ones")
            nc.vector.memset(ones[:], 1.0)
            pg = psump.tile([10, GPC], F32, space="PSUM", name="pg")
            for g in range(GPC):
                for j in range(CPG):
                    c = g * CPG + j
                    nc.tensor.matmul(pg[:, g:g + 1], lhsT=xn[:, c * 10:(c + 1) * 10],
                                     rhs=ones[:], start=(j == 0), stop=(j == CPG - 1))
            pooled_t = pool.tile([10, GPC], F32, name="pooledt")
            nc.vector.tensor_scalar_mul(pooled_t[:], pg[:], 1.0 / (NPER // 4))
            ptr = psump.tile([GPC, 10], F32, space="PSUM", name="ptr")
            nc.tensor.transpose(out=ptr[:], in_=pooled_t[:], identity=ident[:10, :10])
            pooled = pool.tile([GPC, 10], F32, name="pooled")
            nc.vector.tensor_copy(pooled[:], ptr[:])
            m = pool.tile([GPC, 2], F32, name="m")
            nc.vector.tensor_reduce(m[:, 0:1], pooled[:], axis=AX.X, op=ALU.max)
            nc.vector.tensor_scalar_mul(m[:, 1:2], m[:, 0:1], -1.0)
            ex = pool.tile([GPC, 10], F32, name="ex")
            se = pool.tile([GPC, 1], F32, name="se")
            nc.scalar.activation(ex[:], pooled[:], AF.Exp, bias=m[:, 1:2], accum_out=se[:])
            lse = pool.tile([GPC, 1], F32, name="lse")
            nc.scalar.activation(lse[:], se[:], AF.Ln)
            o = pool.tile([GPC, 10], F32, name="o")
            nc.vector.tensor_scalar(o[:], pooled[:], scalar1=m[:, 1:2], scalar2=None,
                                    op0=ALU.add)
            nc.vector.tensor_scalar(o[:], o[:], scalar1=lse[:], scalar2=None,
                                    op0=ALU.subtract)
            nc.sync.dma_start(out_d.ap(), o[:])
    nc.compile()
    return nc


# ===== top-level driver =====
from concourse import bass_utils

_CACHE = {}


def _get_programs(Kw, reps=1):
    key = (tuple(int(k) for k in Kw), reps)
    if key not in _CACHE:
        _CACHE[key] = (build_k0(), build_k1(Kw, reps=reps), build_k2(Kw, reps=reps),
                       build_k3())
    return _CACHE[key]


def _run(nc, feeds):
    res = bass_utils.run_bass_kernel_spmd(nc, feeds, core_ids=list(range(NCORES)))
    return res.results


def make_feeds(inputs, st):
    """Build the per-launch feed dicts (K2/K3 feeds need device outputs patched in)."""
    pk = st["packs"]
    feeds0 = [dict(xT=st["percore"][c]["xT"]) for c in range(NCORES)]
    common1 = dict(gamma=np.asarray(inputs["gamma"], np.float32).reshape(128, 1),
                   beta=np.asarray(inputs["beta"], np.float32).reshape(128, 1),
                   W1=inputs["W1"], W1T=np.ascontiguousarray(inputs["W1"].T),
                   W2=inputs["W2"], W2T=np.ascontiguousarray(inputs["W2"].T),
                   asf1=pk["asf1"], adf1=pk["adf1"], asf2=pk["asf2"], adf2=pk["adf2"],
                   b1=inputs["b1"].reshape(1, -1), b2=inputs["b2"].reshape(1, -1),
                   p1=inputs["p1"].reshape(1, -1))
    common2 = dict(W3=inputs["W3"], W3T=np.ascontiguousarray(inputs["W3"].T),
                   W4=inputs["W4"], W4T=np.ascontiguousarray(inputs["W4"].T),
                   asf3=pk["asf3"], adf3=pk["adf3"], asf4=pk["asf4"], adf4=pk["adf4"],
                   b3=inputs["b3"].reshape(1, -1), b4=inputs["b4"].reshape(1, -1),
                   p2=inputs["p2"].reshape(1, -1))
    return feeds0, common1, common2


def kernel(**inputs):
    inputs = {k: np.asarray(v) for k, v in inputs.items()}
    st = prep_static(inputs)
    Kw = st["K"]
    nc0, nc1, nc2, nc3 = _get_programs(Kw)
    feeds0, common1, common2 = make_feeds(inputs, st)

    o0 = _run(nc0, feeds0)
    stats = np.sum([o["stats"] for o in o0], axis=0).astype(np.float32)

    feeds1 = [dict(xT=st["percore"][c]["xT"], idx=st["percore"][c]["idx16"],
                   stats=stats, **common1) for c in range(NCORES)]
    o1 = _run(nc1, feeds1)

    sc1 = [o["score1"] for o in o1]
    nm1 = host_topk(sc1, st["perms"], None, NPER // 2)

    feeds2 = [dict(x3T=o1[c]["x3T"], score1=sc1[c], nm1=nm1[c],
                   idx=st["percore"][c]["idx16"], **common2) for c in range(NCORES)]
    o2 = _run(nc2, feeds2)

    sc2 = [o["score2"] for o in o2]
    nm2 = host_topk(sc2, st["perms"], nm1, NPER // 4)

    feeds3 = [dict(x5=o2[c]["x5"], score2=sc2[c], nm2=nm2[c]) for c in range(NCORES)]
    o3 = _run(nc3, feeds3)
    out = np.concatenate([o["out"] for o in o3], axis=0).astype(np.float32)
    return out



# revision 7
# speedup vs baseline: 17.7560x; 17.7560x over previous
"""Trainium2 Bass kernel for nn_GAT_77953656422757 (GATConv x4 + TopKPool x2).

Sharding: graph-level data parallel, 4 graphs per NeuronCore x 8 cores.
Pipeline: K0 (BN stats) -> K1 (BN+GAT1+GAT2+score1) -> host top-k ->
K2 (pool-scale+GAT3+GAT4+score2) -> host top-k -> K3 (pool+mean+log_softmax).
Attention uses the exact factorization exp(leaky_relu(a+b)) = max(e^a e^b, e^{a/5} e^{b/5})
so per-edge work is two multiplies and a max (custom fused DVE op), with
per-node exp tables and an all-zero sentinel row for padding.

v2: bf16 256B table rows, gathers spread over 4 SWDGE queues with up to
2048 idxs/call, per-graph produce->gather barriers, Act-engine attention
broadcast, runtime repeat-count input for low-noise timing.
"""
import sys
sys.path.insert(0, "/opt/trn_rl_repo")
import numpy as np

B = 32
NPER = 2048
DEG = 16
NT = B * NPER
NF = 128
NCLS = 10
NCORES = 8
GPC = B // NCORES          # graphs per core = 4
NPCORE = GPC * NPER        # nodes per core = 8192
NCHUNK = NPCORE // 128     # 64 chunks per core
JCHUNK = NPER // 128       # 16 rank-slices per graph

SENT = NPCORE              # sentinel row index (per-core table has NPCORE+1 rows)
NCH2 = 32                  # compacted chunks per core after pool1
NA = 4096                  # active nodes per core after pool1
NAPG = 1024                # active nodes per graph after pool1
JCH2 = 8                   # rank-slices per graph (compacted)
SENT2 = NA
RMAX = 8                   # max slots (128-idx groups) per dma_gather call (1024-desc ring)
NQ = 4                     # SWDGE queues
SCRATCH = 16384            # dynamic DMA scratch (ring = SCRATCH/16 descriptors)


# ===================================================================== host


def build_core_graphs(edge_index):
    """Split the global edge list into per-core local edge lists (with self-loops)."""
    src_g = np.asarray(edge_index[0])
    dst_g = np.asarray(edge_index[1])
    E_per_graph = NPER * DEG
    cores = []
    for c in range(NCORES):
        base_node = c * NPCORE
        e0 = c * GPC * E_per_graph
        e1 = (c + 1) * GPC * E_per_graph
        src = src_g[e0:e1] - base_node
        dst = dst_g[e0:e1] - base_node
        sl = np.arange(NPCORE, dtype=np.int32)
        src = np.concatenate([src, sl]).astype(np.int64)
        dst = np.concatenate([dst, sl]).astype(np.int64)
        cores.append((src, dst))
    return cores


def degree_sort_perms(cores):
    """Per-core permutation: within each graph, nodes sorted by in-degree desc.
    perm[new_local_id] = old_local_id."""
    perms = []
    for (src, dst) in cores:
        deg = np.bincount(dst, minlength=NPCORE)
        perm = np.empty(NPCORE, dtype=np.int64)
        for g in range(GPC):
            lo, hi = g * NPER, (g + 1) * NPER
            order = np.argsort(-deg[lo:hi], kind="stable")
            perm[lo:hi] = lo + order
        perms.append(perm)
    return perms


def chunk_widths(cores, perms):
    """K_j for j in [0, JCHUNK): max (over all cores+graphs) in-degree at
    rank-slice j, so the compiled program is identical across cores."""
    K = np.zeros(JCHUNK, dtype=np.int64)
    for (src, dst), perm in zip(cores, perms):
        deg = np.bincount(dst, minlength=NPCORE)
        deg_sorted = deg[perm]
        for g in range(GPC):
            d = deg_sorted[g * NPER:(g + 1) * NPER]
            for j in range(JCHUNK):
                K[j] = max(K[j], d[j * 128:(j + 1) * 128].max())
    K = ((K + 1) // 2) * 2
    return K


def gather_groups_w(Kc, nchunk):
    """Static per-chunk gather splitting: per chunk list of (slot_off, r)."""
    groups = []
    for c in range(nchunk):
        k = int(Kc[c])
        g, s = [], 0
        while s < k:
            r = min(RMAX, k - s)
            g.append((s, r))
            s += r
        groups.append(g)
    return groups


def goffs_of_w(Kc, nchunk):
    """Static gather-group column offsets in the idx16 table."""
    groups = gather_groups_w(Kc, nchunk)
    goffs, w = [], 0
    for c in range(nchunk):
        go = []
        for (s0, r) in groups[c]:
            go.append((w, s0, r))
            w += 8 * r
        goffs.append(go)
    return goffs, w


def build_idx_tables_generic(src, dst, n_nodes, sent, Kc, nchunk):
    """Slot tables for a (src, dst) edge list over n_nodes (already in
    new-id order). Kc: per-chunk widths [nchunk].
    Returns (idx_cm int32 [128, S_tot], idx16 int16 [128, W_tot])."""
    order = np.argsort(dst, kind="stable")
    src_s = src[order]
    deg = np.bincount(dst, minlength=n_nodes)
    starts = np.concatenate([[0], np.cumsum(deg)])
    S_tot = int(sum(int(Kc[c]) for c in range(nchunk)))
    idx_cm = np.full((128, S_tot), sent, dtype=np.int32)
    off = 0
    offs = []
    for c in range(nchunk):
        k = int(Kc[c])
        offs.append(off)
        for p in range(128):
            d = 128 * c + p
            s, e = starts[d], starts[d + 1]
            assert e - s <= k, (c, p, e - s, k)
            idx_cm[p, off:off + (e - s)] = src_s[s:e]
        off += k
    groups = gather_groups_w(Kc, nchunk)
    W_tot = sum(8 * r for g in groups for (_, r) in g)
    idx16 = np.zeros((128, W_tot), np.int16)
    w = 0
    for c in range(nchunk):
        for (s0, r) in groups[c]:
            ni = 128 * r
            lin = np.empty(ni, np.int64)
            for k in range(r):
                lin[k * 128:(k + 1) * 128] = idx_cm[:, offs[c] + s0 + k]
            blk = np.tile(lin.reshape(ni // 16, 16).T, (8, 1)).astype(np.int16)
            idx16[:, w:w + ni // 16] = blk
            w += ni // 16
    return idx_cm, idx16, offs


def build_idx_tables(core, perm, K):
    """Per-core K1 slot tables (all 64 chunks, width pattern K[j%16])."""
    src, dst = core
    inv = np.empty(NPCORE, dtype=np.int64)
    inv[perm] = np.arange(NPCORE)
    src_n = inv[src]
    dst_n = inv[dst]
    Kc = [int(K[c % JCHUNK]) for c in range(NCHUNK)]
    return build_idx_tables_generic(src_n, dst_n, NPCORE, SENT, Kc, NCHUNK)


def leaky(x, a=0.2):
    return np.where(x > 0, x, a * x)


def pack_asf(a, H, C):
    m = np.zeros((H * C, H), np.float32)
    for h in range(H):
        m[h * C:(h + 1) * C, h] = a[h]
    return m


def prep_static(inputs):
    """All input-independent-of-device-results preprocessing, per core."""
    x = np.asarray(inputs["x"], np.float32)
    edge_index = np.asarray(inputs["edge_index"])
    cores = build_core_graphs(edge_index)
    perms = degree_sort_perms(cores)
    K = chunk_widths(cores, perms)
    percore = []
    for c in range(NCORES):
        idx_cm, idx16, offs = build_idx_tables(cores[c], perms[c], K)
        xc = x[c * NPCORE:(c + 1) * NPCORE][perms[c]]
        xT = np.ascontiguousarray(xc.T)
        percore.append(dict(idx_cm=idx_cm, idx16=idx16, offs=offs,
                            xT=xT, perm=perms[c]))
    packs = dict(
        asf1=pack_asf(np.asarray(inputs["as1"]), 3, 16), adf1=pack_asf(np.asarray(inputs["ad1"]), 3, 16),
        asf2=pack_asf(np.asarray(inputs["as2"]), 3, 16), adf2=pack_asf(np.asarray(inputs["ad2"]), 3, 16),
        asf3=pack_asf(np.asarray(inputs["as3"]), 2, 16), adf3=pack_asf(np.asarray(inputs["ad3"]), 2, 16),
        asf4=pack_asf(np.asarray(inputs["as4"]), 1, 10), adf4=pack_asf(np.asarray(inputs["ad4"]), 1, 10),
    )
    return dict(cores=cores, perms=perms, K=K, percore=percore, packs=packs)


def host_topk(score_cm_list, perms, nm_prev_cm_list, k):
    """score_cm [128, NCHUNK] per core -> nm_cm [128, NCHUNK] per core, exact
    reference tie semantics (original-id order)."""
    out = []
    for c in range(NCORES):
        score = score_cm_list[c].T.reshape(-1)
        nm_prev = nm_prev_cm_list[c].T.reshape(-1) if nm_prev_cm_list else np.ones(NPCORE, np.float32)
        perm = perms[c]
        nm = np.zeros(NPCORE, np.float32)
        for g in range(GPC):
            sl = slice(g * NPER, (g + 1) * NPER)
            s_orig = np.empty(NPER, np.float32)
            so = score[sl].copy()
            so[nm_prev[sl] == 0] = -1e9
            s_orig[perm[sl] - g * NPER] = so
            keep = np.argsort(-s_orig, kind="stable")[:k]
            km = np.zeros(NPER, np.float32)
            km[keep] = 1.0
            nm[sl] = km[perm[sl] - g * NPER] * nm_prev[sl]
        out.append(np.ascontiguousarray(nm.reshape(NCHUNK, 128).T))
    return out


def prep_compact(st, nm1_list):
    """Compacted per-core layout for K2: 4096 active nodes (1024/graph),
    degree-sorted within graph, edges restricted to active-active pairs."""
    cores, perms = st["cores"], st["perms"]
    KC = np.zeros(JCH2, np.int64)
    tmp = []
    for c in range(NCORES):
        src, dst = cores[c]
        perm = perms[c]
        inv = np.empty(NPCORE, np.int64)
        inv[perm] = np.arange(NPCORE)
        src_n = inv[src]
        dst_n = inv[dst]
        nm = nm1_list[c].T.reshape(-1) > 0
        both = nm[src_n] & nm[dst_n]
        sb, db = src_n[both], dst_n[both]
        deg = np.bincount(db, minlength=NPCORE)
        cid = np.full(NPCORE, -1, np.int64)
        x3sel = np.empty(NA, np.int64)
        for g in range(GPC):
            lo = g * NPER
            ids = np.where(nm[lo:lo + NPER])[0] + lo
            order = np.argsort(-deg[ids], kind="stable")
            sel = ids[order]
            cid[sel] = g * NAPG + np.arange(NAPG)
            x3sel[g * NAPG:(g + 1) * NAPG] = sel
            dd = deg[sel]
            for j in range(JCH2):
                KC[j] = max(KC[j], dd[j * 128:(j + 1) * 128].max())
        tmp.append((sb, db, cid, x3sel))
    KC = ((KC + 1) // 2) * 2
    Kc2 = [int(KC[cc % JCH2]) for cc in range(NCH2)]
    percore = []
    for c in range(NCORES):
        sb, db, cid, x3sel = tmp[c]
        idx_cm, idx16, offs = build_idx_tables_generic(
            cid[sb], cid[db], NA, SENT2, Kc2, NCH2)
        percore.append(dict(idx16=idx16, x3sel=x3sel, cid=cid))
    return dict(KC=KC, percore=percore)


def host_topk2(sc2c_list, perms, percore_c, k):
    """Compact-order scores -> compact-order keep mask [128, NCH2] per core,
    exact reference tie semantics."""
    out = []
    for c in range(NCORES):
        score = sc2c_list[c].T.reshape(-1)
        x3sel = percore_c[c]["x3sel"]
        perm = perms[c]
        nm = np.zeros(NA, np.float32)
        for g in range(GPC):
            sl = slice(g * NAPG, (g + 1) * NAPG)
            s_orig = np.full(NPER, -1e9, np.float32)
            newids = x3sel[sl]
            origpos = perm[newids] - g * NPER
            s_orig[origpos] = score[sl]
            keep = np.argsort(-s_orig, kind="stable")[:k]
            km = np.zeros(NPER, np.float32)
            km[keep] = 1.0
            nm[sl] = km[origpos]
        out.append(np.ascontiguousarray(nm.reshape(NCH2, 128).T))
    return out


# ===== custom DVE op =====
from concourse.dve_ops import OPS, DveOp, _SUB_OPCODE_FOR_NAME, _CUSTOM_DVE_ROW_BASE
from concourse.dve_spec import Spec, Src0, Src1, C0, C1, Zero, maxx, lower, _has_src1
from concourse.dve_uop import DveOpSpec
from concourse.dve_table_gen import dve_ver_for
from operator import add


def _ref_ex_den(in0, in1, s0, s1, imm2):
    b = np.maximum(in0.astype(np.float32) * s0, in1.astype(np.float32) * s1).astype(
        np.float32
    )
    return b, b.reshape(b.shape[0], -1).sum(axis=-1, keepdims=True).astype(np.float32)


def register_ex_den():
    name = "GAT_EX_DEN_ANT"
    for op in OPS:
        if op.name == name:
            return op
    spec = Spec(
        body=maxx(Src0 * C0, Src1 * C1),
        accum=add,
        accum_init=Zero,
        reference=_ref_ex_den,
    )
    op = DveOp(name, spec, subdim=False, uops_sha={})
    OPS.append(op)
    _SUB_OPCODE_FOR_NAME[name] = _CUSTOM_DVE_ROW_BASE + len(OPS) - 1
    from concourse.dve_ops import CUSTOM_DVE_SPECS
    CUSTOM_DVE_SPECS[name] = spec
    for ver in ("v3",):
        tmp = DveOpSpec(
            name=name,
            opcode=_SUB_OPCODE_FOR_NAME[name],
            uops=lower(spec, ver=ver),
            rd1_en=_has_src1(spec),
        )
        op.uops_sha[ver] = tmp.sha(ver)
    return op


EX_DEN = register_ex_den()


# ===== kernels =====

import concourse.bass as bass
import concourse.bacc as bacc
import concourse.mybir as mybir
import concourse.tile as tile
from concourse import library_config
from concourse.tile_rust import add_dep_helper
from concourse.masks import make_identity
from concourse.dve_ops import TENSOR_TENSOR_REDUCE

F32 = mybir.dt.float32
BF16 = mybir.dt.bfloat16
I32 = mybir.dt.int32
I16 = mybir.dt.int16
AF = mybir.ActivationFunctionType
ALU = mybir.AluOpType
AX = mybir.AxisListType

ROWW = 128   # bf16 elements per 256B table row
LCFG = {
    1: dict(Cin=128, Cout=48, H=3),
    2: dict(Cin=48, Cout=48, H=3),
    3: dict(Cin=48, Cout=32, H=2),
    4: dict(Cin=32, Cout=10, H=1),
}


def new_nc():
    return bacc.Bacc("TRN2", target_bir_lowering=False, debug=False, num_devices=8,
                     num_swdge_queues=NQ, dynamic_dma_scratch_size=SCRATCH)


# ---------------------------------------------------------------- K0: BN stats
def build_k0():
    nc = bacc.Bacc("TRN2", target_bir_lowering=False, debug=False, num_devices=8)
    xT = nc.dram_tensor("xT", [128, NPCORE], F32, kind="ExternalInput")
    stats = nc.dram_tensor("stats", [128, 2], F32, kind="ExternalOutput")
    with tile.TileContext(nc) as tc:
        with tc.tile_pool(name="p", bufs=1) as pool:
            xt = pool.tile([128, NPCORE], F32)
            nc.sync.dma_start(xt[:], xT.ap())
            sq = pool.tile([128, NPCORE], F32)
            nc.scalar.activation(sq[:], xt[:], AF.Square)
            st = pool.tile([128, 2], F32)
            nc.vector.tensor_reduce(st[:, 0:1], xt[:], axis=AX.X, op=ALU.add)
            nc.vector.tensor_reduce(st[:, 1:2], sq[:], axis=AX.X, op=ALU.add)
            nc.sync.dma_start(stats.ap(), st[:])
    nc.compile()
    return nc


# ------------------------------------------------------------ shared emitters
class LayerCtx:
    def __init__(self, nc, tc, pools, ident):
        self.nc, self.tc, self.pools, self.ident = nc, tc, pools, ident
        self.qctr = 0

    def next_q(self):
        q = self.qctr % NQ
        self.qctr += 1
        return q


def emit_reps_loop(nc, tc, resp, reps, body):
    """Repeat body() `reps` times via a hardware loop (static bound)."""
    if reps > 1:
        with tc.For_i(0, reps) as _:
            body()
    else:
        body()


def emit_wfull(nc, pools, W_ap, WT_ap, asf_ap, adf_ap, Cin, Cout, H, name):
    """Build Wfull [Cin, Cout+2H] sbuf tile = [W | W@asf | W@adf]."""
    wf = pools["const"].tile([Cin, Cout + 2 * H], F32, name=f"wf{name}")
    nc.sync.dma_start(wf[:, 0:Cout], W_ap)
    ps = pools["psum"].tile([Cin, 2 * H], F32, space="PSUM", name=f"wps{name}", tag="pp")
    nc.tensor.matmul(ps[:, 0:H], lhsT=WT_ap, rhs=asf_ap, start=True, stop=True)
    nc.tensor.matmul(ps[:, H:2 * H], lhsT=WT_ap, rhs=adf_ap, start=True, stop=True)
    nc.vector.tensor_copy(wf[:, Cout:Cout + 2 * H], ps[:])
    return wf


def emit_produce(ctx, li, c, xT_chunk_ap, wf, T_dram, vv, hbias=None, scale_col=None,
                 nm_col=None, trow_writes=None):
    """Produce table rows for chunk c of layer li and write to T_dram (bf16)."""
    nc, pools = ctx.nc, ctx.pools
    cfg = LCFG[li]
    Cout, H = cfg["Cout"], cfg["H"]
    WF = Cout + 2 * H
    ps = pools["psum"].tile([128, WF], F32, space="PSUM", name=f"pps{li}", tag="pp")
    nc.tensor.matmul(ps[:], lhsT=xT_chunk_ap, rhs=wf[:], start=True, stop=True)
    srow = pools["srow"].tile([128, WF], F32, name=f"srow{li}", tag="srow")
    if hbias is not None:
        nc.vector.tensor_tensor(srow[:], ps[:], hbias, op=ALU.add)
    elif scale_col is not None:
        nc.vector.tensor_scalar_mul(srow[:], ps[:], scale_col)
    else:
        nc.vector.tensor_copy(srow[:], ps[:])
    trow = pools["trow"].tile([128, ROWW], BF16, name=f"trow{li}", tag="trow")
    nc.vector.memset(trow[:, Cout + 2 * H:ROWW], 0)
    nc.vector.tensor_copy(trow[:, 0:Cout], srow[:, 0:Cout])
    # u = exp(als), u2 = exp(0.2 als)
    nc.scalar.activation(trow[:, Cout:Cout + H], srow[:, Cout:Cout + H], AF.Exp)
    nc.scalar.activation(trow[:, Cout + H:Cout + 2 * H], srow[:, Cout:Cout + H],
                         AF.Exp, scale=0.2)
    # v = exp(ald), v2 = exp(0.2 ald) -> resident vv [128, nchunk, 2H] f32
    nc.scalar.activation(vv[:, c, 0:H], srow[:, Cout + H:Cout + 2 * H], AF.Exp)
    nc.scalar.activation(vv[:, c, H:2 * H], srow[:, Cout + H:Cout + 2 * H],
                         AF.Exp, scale=0.2)
    if nm_col is not None:
        nc.vector.tensor_scalar_mul(trow[:, Cout:Cout + 2 * H],
                                    trow[:, Cout:Cout + 2 * H], nm_col)
        nc.vector.tensor_scalar_mul(vv[:, c, 0:2 * H], vv[:, c, 0:2 * H], nm_col)
    w = nc.sync.dma_start(T_dram.ap()[c * 128:(c + 1) * 128, :], trow[:])
    if trow_writes is not None:
        trow_writes.append(w)


def emit_attention(ctx, li, c, K, groups, idx_tile, T_dram, vv, brow, gather_dep=None):
    """Attention for chunk c of layer li; returns rows tile [128, Cout] f32."""
    nc, pools = ctx.nc, ctx.pools
    cfg = LCFG[li]
    Cout, H = cfg["Cout"], cfg["H"]
    CperH = Cout // H
    gbuf = pools["gbuf"].tile([128, K * ROWW], BF16, name=f"gbuf{li}", tag="gbuf")
    for (w0, s0, r) in groups:
        gi = nc.gpsimd.dma_gather(
            gbuf[:, s0 * ROWW:(s0 + r) * ROWW].rearrange("p (k r) -> p k r", r=ROWW),
            T_dram.ap(), idx_tile[:, w0:w0 + 8 * r], 128 * r, 128 * r, ROWW,
            queue_num=ctx.next_q())
        for dep in (gather_dep or []):
            add_dep_helper(gi.ins, dep.ins, True, "dep before gather")
    g32 = gbuf[:].rearrange("p (k r) -> p k r", r=ROWW)
    ex = pools["ex"].tile([128, H * K], BF16, name=f"ex{li}", tag="ex")
    for h in range(H):
        nc.vector._custom_dve(
            EX_DEN, out=ex[:, h * K:(h + 1) * K],
            in0=g32[:, :, Cout + h], in1=g32[:, :, Cout + H + h],
            s0=vv[:, c, h:h + 1], s1=vv[:, c, H + h:H + h + 1])
    den = pools["den"].tile([128, H], F32, name=f"den{li}", tag="den")
    nc.vector.tensor_reduce(den[:], ex[:].rearrange("p (h k) -> p h k", h=H),
                            axis=AX.X, op=ALU.add)
    rden = pools["den"].tile([128, H], F32, name=f"rden{li}", tag="rden")
    nc.vector.tensor_scalar_max(rden[:], den[:], 1e-16)
    nc.vector.reciprocal(rden[:], rden[:])
    # exh[p,k,h,cc] = ex[p,h,k] broadcast over cc, on the Act engine
    exh = pools["tmp"].tile([128, K * Cout], BF16, name=f"exh{li}", tag="exh")
    ex4 = ex[:].rearrange("p (h k) -> p k h", h=H).unsqueeze(3).broadcast_to(
        [128, K, H, CperH])
    nc.scalar.activation(
        exh[:].rearrange("p (k h cc) -> p k h cc", h=H, cc=CperH), ex4, AF.Copy)
    # tmp = h_g * exh  (all-bf16 packed -> 2x DVE)
    tmp = pools["tmp"].tile([128, K * Cout], BF16, name=f"tmp{li}", tag="tmp")
    nc.vector.tensor_tensor(
        tmp[:].rearrange("p (k f) -> p k f", f=Cout), g32[:, :, 0:Cout],
        exh[:].rearrange("p (k f) -> p k f", f=Cout), op=ALU.mult)
    pre = pools["pre"].tile([128, Cout], F32, name=f"pre{li}", tag="pre")
    nc.vector.tensor_reduce(pre[:], tmp[:].rearrange("p (k f) -> p f k", f=Cout),
                            axis=AX.X, op=ALU.add)
    rows = pools["rows"].tile([128, Cout], F32, name=f"rows{li}", tag="rows")
    rden3 = rden[:].unsqueeze(2).broadcast_to([128, H, CperH])
    nc.vector.tensor_tensor(rows[:].rearrange("p (h cc) -> p h cc", h=H),
                            pre[:].rearrange("p (h cc) -> p h cc", h=H), rden3,
                            op=ALU.mult)
    nc.vector.tensor_tensor(rows[:], rows[:], brow, op=ALU.add)
    return rows


def emit_transpose(ctx, rows, Cout, dst_ap):
    nc, pools = ctx.nc, ctx.pools
    pt = pools["psum"].tile([Cout, 128], F32, space="PSUM", name="ptr", tag="ptr")
    nc.tensor.transpose(out=pt[:], in_=rows[:], identity=ctx.ident[:])
    nc.vector.tensor_copy(dst_ap, pt[:])


def emit_rnorm(nc, pools, p_tile, C):
    sq = pools["const"].tile([128, C], F32, name=f"pn{C}")
    nc.vector.tensor_tensor(sq[:], p_tile[:], p_tile[:], op=ALU.mult)
    nrm = pools["const"].tile([128, 2], F32, name=f"nrm{C}")
    nc.vector.tensor_reduce(nrm[:, 0:1], sq[:], axis=AX.X, op=ALU.add)
    nc.scalar.activation(nrm[:, 1:2], nrm[:, 0:1], AF.Sqrt)
    nc.vector.reciprocal(nrm[:, 0:1], nrm[:, 1:2])
    return nrm


def emit_score(ctx, rows, Cout, p_tile, rnorm, score_out_col):
    nc = ctx.nc
    sctmp = ctx.pools["pre"].tile([128, Cout], F32, name="sctmp", tag="sctmp")
    nc.vector._custom_dve(
        TENSOR_TENSOR_REDUCE, out=sctmp[:],
        in0=rows[:], in1=p_tile[:],
        s0=0.0, s1=rnorm[:, 0:1],
        accum_out=score_out_col)


def emit_graph_barriers(nc, constp, writes_per_graph, sent_w, name):
    """One DVE barrier tile per graph gated on that graph's T writes."""
    bars = []
    for g, ws in enumerate(writes_per_graph):
        bar = nc.vector.memset(constp.tile([1, 1], F32, name=f"bar{name}{g}")[:], 0)
        add_dep_helper(bar.ins, sent_w.ins, True, "sentinel before gathers")
        for w in ws:
            add_dep_helper(bar.ins, w.ins, True, "T writes before gathers")
        bars.append(bar)
    return bars


# ------------------------------------------------------------------- K1
def build_k1(Kw, reps=1):
    """BN + L1 + L2 + score1.  Kw = per-rank-slice widths [16]."""
    Kc = [int(Kw[c % JCHUNK]) for c in range(NCHUNK)]
    goffs, W_tot = goffs_of_w(Kc, NCHUNK)
    nc = new_nc()
    xT_d = nc.dram_tensor("xT", [128, NPCORE], F32, kind="ExternalInput")
    stats_d = nc.dram_tensor("stats", [128, 2], F32, kind="ExternalInput")
    gamma_d = nc.dram_tensor("gamma", [128, 1], F32, kind="ExternalInput")
    beta_d = nc.dram_tensor("beta", [128, 1], F32, kind="ExternalInput")
    W1_d = nc.dram_tensor("W1", [128, 48], F32, kind="ExternalInput")
    W1T_d = nc.dram_tensor("W1T", [48, 128], F32, kind="ExternalInput")
    W2_d = nc.dram_tensor("W2", [48, 48], F32, kind="ExternalInput")
    W2T_d = nc.dram_tensor("W2T", [48, 48], F32, kind="ExternalInput")
    asf1_d = nc.dram_tensor("asf1", [48, 3], F32, kind="ExternalInput")
    adf1_d = nc.dram_tensor("adf1", [48, 3], F32, kind="ExternalInput")
    asf2_d = nc.dram_tensor("asf2", [48, 3], F32, kind="ExternalInput")
    adf2_d = nc.dram_tensor("adf2", [48, 3], F32, kind="ExternalInput")
    b1_d = nc.dram_tensor("b1", [1, 48], F32, kind="ExternalInput")
    b2_d = nc.dram_tensor("b2", [1, 48], F32, kind="ExternalInput")
    p1_d = nc.dram_tensor("p1", [1, 48], F32, kind="ExternalInput")
    idx_d = nc.dram_tensor("idx", [128, W_tot], I16, kind="ExternalInput")
    T1_d = nc.dram_tensor("T1", [NPCORE + 1, ROWW], BF16, kind="Internal")
    T2_d = nc.dram_tensor("T2", [NPCORE + 1, ROWW], BF16, kind="Internal")
    x3T_d = nc.dram_tensor("x3T", [48, NPCORE], F32, kind="ExternalOutput")
    sc1_d = nc.dram_tensor("score1", [128, NCHUNK], F32, kind="ExternalOutput")

    with tile.TileContext(nc) as tc:
        with (
            tc.tile_pool(name="const", bufs=1) as constp,
            tc.tile_pool(name="psum", bufs=4, space="PSUM") as psump,
            tc.tile_pool(name="xin", bufs=4) as xinp,
            tc.tile_pool(name="srow", bufs=4) as srowp,
            tc.tile_pool(name="trow", bufs=4) as trowp,
            tc.tile_pool(name="gbuf", bufs=5) as gbufp,
            tc.tile_pool(name="ex", bufs=8) as exp_,
            tc.tile_pool(name="den", bufs=8) as denp,
            tc.tile_pool(name="tmp", bufs=6) as tmpp,
            tc.tile_pool(name="pre", bufs=4) as prep,
            tc.tile_pool(name="rows", bufs=4) as rowsp,
            tc.tile_pool(name="res", bufs=1) as resp,
        ):
            pools = dict(const=constp, psum=psump, xin=xinp, srow=srowp, trow=trowp,
                         gbuf=gbufp, ex=exp_, den=denp, tmp=tmpp, pre=prep, rows=rowsp)
            ident = constp.tile([128, 128], F32, name="ident")
            make_identity(nc, ident[:])
            ctx = LayerCtx(nc, tc, pools, ident)

            idx_t = resp.tile([128, W_tot], I16, name="idxt")
            nc.sync.dma_start(idx_t[:], idx_d.ap())
            lib = nc.gpsimd.load_library(library_config.mlp)
            x2T = resp.tile([48, NPCORE], F32, name="x2T")
            vv1 = resp.tile([128, NCHUNK, 6], F32, name="vv1")
            vv2_ = resp.tile([128, NCHUNK, 6], F32, name="vv2")
            zrow = constp.tile([1, ROWW], BF16, name="zrow")
            nc.vector.memset(zrow[:], 0)
            gam = constp.tile([128, 1], F32, name="gam")
            nc.sync.dma_start(gam[:], gamma_d.ap())
            bet = constp.tile([128, 1], F32, name="bet")
            nc.sync.dma_start(bet[:], beta_d.ap())
            stats_t = constp.tile([128, 2], F32, name="stats")
            nc.sync.dma_start(stats_t[:], stats_d.ap())
            b1_t = constp.tile([128, 48], F32, name="b1")
            nc.sync.dma_start(b1_t[:], b1_d.ap().to_broadcast([128, 48]))
            b2_t = constp.tile([128, 48], F32, name="b2")
            nc.sync.dma_start(b2_t[:], b2_d.ap().to_broadcast([128, 48]))
            p1_t = constp.tile([128, 48], F32, name="p1")
            nc.sync.dma_start(p1_t[:], p1_d.ap().to_broadcast([128, 48]))
            asf1_t = constp.tile([48, 3], F32, name="asf1")
            nc.sync.dma_start(asf1_t[:], asf1_d.ap())
            adf1_t = constp.tile([48, 3], F32, name="adf1")
            nc.sync.dma_start(adf1_t[:], adf1_d.ap())
            asf2_t = constp.tile([48, 3], F32, name="asf2")
            nc.sync.dma_start(asf2_t[:], asf2_d.ap())
            adf2_t = constp.tile([48, 3], F32, name="adf2")
            nc.sync.dma_start(adf2_t[:], adf2_d.ap())
            w1t_t = constp.tile([48, 128], F32, name="w1t")
            nc.sync.dma_start(w1t_t[:], W1T_d.ap())
            w2t_t = constp.tile([48, 48], F32, name="w2t")
            nc.sync.dma_start(w2t_t[:], W2T_d.ap())

            def body(it=None):
                s1w = nc.sync.dma_start(T1_d.ap()[NPCORE:NPCORE + 1, :], zrow[:])
                s2w = nc.sync.dma_start(T2_d.ap()[NPCORE:NPCORE + 1, :], zrow[:])
                # BN fold
                ab = constp.tile([128, 6], F32, name="ab")
                nc.vector.tensor_scalar_mul(ab[:, 0:2], stats_t[:], 1.0 / NT)
                nc.vector.tensor_tensor(ab[:, 2:3], ab[:, 0:1], ab[:, 0:1], op=ALU.mult)
                nc.vector.tensor_tensor(ab[:, 2:3], ab[:, 1:2], ab[:, 2:3], op=ALU.subtract)
                nc.vector.tensor_scalar_add(ab[:, 2:3], ab[:, 2:3], 1e-5)
                nc.scalar.activation(ab[:, 3:4], ab[:, 2:3], AF.Sqrt)
                nc.vector.reciprocal(ab[:, 3:4], ab[:, 3:4])
                nc.vector.tensor_tensor(ab[:, 4:5], gam[:], ab[:, 3:4], op=ALU.mult)
                nc.vector.tensor_tensor(ab[:, 5:6], ab[:, 0:1], ab[:, 4:5], op=ALU.mult)
                nc.vector.tensor_tensor(ab[:, 5:6], bet[:], ab[:, 5:6], op=ALU.subtract)

                wf1 = emit_wfull(nc, pools, W1_d.ap(), w1t_t[:], asf1_t[:], adf1_t[:], 128, 48, 3, "1")
                nc.vector.tensor_scalar_mul(wf1[:], wf1[:], ab[:, 4:5])
                hb_ps = psump.tile([1, 54], F32, space="PSUM", name="hbps", tag="pp")
                nc.tensor.matmul(hb_ps[:], lhsT=ab[:, 5:6], rhs=wf1[:], start=True, stop=True)
                hb1r = constp.tile([1, 54], F32, name="hb1r")
                nc.vector.tensor_copy(hb1r[:], hb_ps[:])
                onesrow = constp.tile([1, 128], F32, name="onesrow")
                nc.vector.memset(onesrow[:], 1.0)
                hbb_ps = psump.tile([128, 54], F32, space="PSUM", name="hbbps", tag="pp")
                nc.tensor.matmul(hbb_ps[:], lhsT=onesrow[:], rhs=hb1r[:], start=True, stop=True)
                hb1 = constp.tile([128, 54], F32, name="hb1")
                nc.vector.tensor_copy(hb1[:], hbb_ps[:])
                wf2 = emit_wfull(nc, pools, W2_d.ap(), w2t_t[:], asf2_t[:], adf2_t[:], 48, 48, 3, "2")
                rn1 = emit_rnorm(nc, pools, p1_t[:], 48)

                # ---- L1 produce (graph-contiguous chunk order)
                t1w = [[] for _ in range(GPC)]
                for c in range(NCHUNK):
                    xt = xinp.tile([128, 128], F32, name="xt")
                    nc.sync.dma_start(xt[:], xT_d.ap()[:, c * 128:(c + 1) * 128])
                    emit_produce(ctx, 1, c, xt[:], wf1, T1_d, vv1[:], hbias=hb1[:],
                                 trow_writes=t1w[c // JCHUNK])
                bars1 = emit_graph_barriers(nc, constp, t1w, s1w, "t1")
                # ---- L1 attention -> x2T
                for c in range(NCHUNK):
                    rows = emit_attention(ctx, 1, c, Kc[c], goffs[c], idx_t, T1_d,
                                          vv1[:], b1_t[:],
                                          gather_dep=[bars1[c // JCHUNK], lib])
                    emit_transpose(ctx, rows, 48, x2T[:, c * 128:(c + 1) * 128])
                # ---- L2 produce
                t2w = [[] for _ in range(GPC)]
                for c in range(NCHUNK):
                    emit_produce(ctx, 2, c, x2T[:, c * 128:(c + 1) * 128], wf2, T2_d,
                                 vv2_[:], trow_writes=t2w[c // JCHUNK])
                bars2 = emit_graph_barriers(nc, constp, t2w, s2w, "t2")
                # ---- L2 attention -> x3T out + score1
                sc = resp.tile([128, NCHUNK], F32, name="sc")
                for c in range(NCHUNK):
                    rows = emit_attention(ctx, 2, c, Kc[c], goffs[c], idx_t, T2_d,
                                          vv2_[:], b2_t[:],
                                          gather_dep=[bars2[c // JCHUNK], lib])
                    x3c = pools["rows"].tile([48, 128], F32, name="x3c", tag="x3c")
                    emit_transpose(ctx, rows, 48, x3c[:])
                    nc.sync.dma_start(x3T_d.ap()[:, c * 128:(c + 1) * 128], x3c[:])
                    emit_score(ctx, rows, 48, p1_t[:], rn1, sc[:, c:c + 1])
                nc.sync.dma_start(sc1_d.ap(), sc[:])

            emit_reps_loop(nc, tc, resp, reps, body)
    nc.compile()
    return nc


# ------------------------------------------------------------- K2 (compacted)
def build_k2(KC, reps=1):
    """pool1-scale + L3 + L4 + score2 on the compacted active-node layout.
    KC = per-rank-slice widths [JCH2] of the compacted graph."""
    Kc = [int(KC[c % JCH2]) for c in range(NCH2)]
    goffs, W_tot = goffs_of_w(Kc, NCH2)
    nc = new_nc()
    x3T_d = nc.dram_tensor("x3T", [48, NA], F32, kind="ExternalInput")
    sc1_d = nc.dram_tensor("score1", [128, NCH2], F32, kind="ExternalInput")
    W3_d = nc.dram_tensor("W3", [48, 32], F32, kind="ExternalInput")
    W3T_d = nc.dram_tensor("W3T", [32, 48], F32, kind="ExternalInput")
    W4_d = nc.dram_tensor("W4", [32, 10], F32, kind="ExternalInput")
    W4T_d = nc.dram_tensor("W4T", [10, 32], F32, kind="ExternalInput")
    asf3_d = nc.dram_tensor("asf3", [32, 2], F32, kind="ExternalInput")
    adf3_d = nc.dram_tensor("adf3", [32, 2], F32, kind="ExternalInput")
    asf4_d = nc.dram_tensor("asf4", [10, 1], F32, kind="ExternalInput")
    adf4_d = nc.dram_tensor("adf4", [10, 1], F32, kind="ExternalInput")
    b3_d = nc.dram_tensor("b3", [1, 32], F32, kind="ExternalInput")
    b4_d = nc.dram_tensor("b4", [1, 10], F32, kind="ExternalInput")
    p2_d = nc.dram_tensor("p2", [1, 10], F32, kind="ExternalInput")
    idx_d = nc.dram_tensor("idx", [128, W_tot], I16, kind="ExternalInput")
    T3_d = nc.dram_tensor("T3", [NA + 1, ROWW], BF16, kind="Internal")
    T4_d = nc.dram_tensor("T4", [NA + 1, ROWW], BF16, kind="Internal")
    x5_d = nc.dram_tensor("x5", [128, NCH2 * 10], F32, kind="ExternalOutput")
    sc2_d = nc.dram_tensor("score2", [128, NCH2], F32, kind="ExternalOutput")

    with tile.TileContext(nc) as tc:
        with (
            tc.tile_pool(name="const", bufs=1) as constp,
            tc.tile_pool(name="psum", bufs=4, space="PSUM") as psump,
            tc.tile_pool(name="srow", bufs=4) as srowp,
            tc.tile_pool(name="trow", bufs=4) as trowp,
            tc.tile_pool(name="gbuf", bufs=5) as gbufp,
            tc.tile_pool(name="ex", bufs=8) as exp_,
            tc.tile_pool(name="den", bufs=8) as denp,
            tc.tile_pool(name="tmp", bufs=6) as tmpp,
            tc.tile_pool(name="pre", bufs=4) as prep,
            tc.tile_pool(name="rows", bufs=4) as rowsp,
            tc.tile_pool(name="res", bufs=1) as resp,
        ):
            pools = dict(const=constp, psum=psump, srow=srowp, trow=trowp,
                         gbuf=gbufp, ex=exp_, den=denp, tmp=tmpp, pre=prep, rows=rowsp)
            ident = constp.tile([128, 128], F32, name="ident")
            make_identity(nc, ident[:])
            ctx = LayerCtx(nc, tc, pools, ident)

            idx_t = resp.tile([128, W_tot], I16, name="idxt")
            nc.sync.dma_start(idx_t[:], idx_d.ap())
            lib = nc.gpsimd.load_library(library_config.mlp)
            x3T = resp.tile([48, NA], F32, name="x3T")
            nc.sync.dma_start(x3T[:], x3T_d.ap())
            sc1_t = resp.tile([128, NCH2], F32, name="sc1")
            nc.sync.dma_start(sc1_t[:], sc1_d.ap())
            x4T = resp.tile([32, NA], F32, name="x4T")
            vv3 = resp.tile([128, NCH2, 4], F32, name="vv3")
            vv4 = resp.tile([128, NCH2, 2], F32, name="vv4")
            zrow = constp.tile([1, ROWW], BF16, name="zrow")
            nc.vector.memset(zrow[:], 0)
            b3_t = constp.tile([128, 32], F32, name="b3")
            nc.sync.dma_start(b3_t[:], b3_d.ap().to_broadcast([128, 32]))
            b4_t = constp.tile([128, 10], F32, name="b4")
            nc.sync.dma_start(b4_t[:], b4_d.ap().to_broadcast([128, 10]))
            p2_t = constp.tile([128, 10], F32, name="p2")
            nc.sync.dma_start(p2_t[:], p2_d.ap().to_broadcast([128, 10]))
            asf3_t = constp.tile([32, 2], F32, name="asf3")
            nc.sync.dma_start(asf3_t[:], asf3_d.ap())
            adf3_t = constp.tile([32, 2], F32, name="adf3")
            nc.sync.dma_start(adf3_t[:], adf3_d.ap())
            asf4_t = constp.tile([10, 1], F32, name="asf4")
            nc.sync.dma_start(asf4_t[:], asf4_d.ap())
            adf4_t = constp.tile([10, 1], F32, name="adf4")
            nc.sync.dma_start(adf4_t[:], adf4_d.ap())
            w3t_t = constp.tile([32, 48], F32, name="w3t")
            nc.sync.dma_start(w3t_t[:], W3T_d.ap())
            w4t_t = constp.tile([10, 32], F32, name="w4t")
            nc.sync.dma_start(w4t_t[:], W4T_d.ap())

            def body(it=None):
                s3w = nc.sync.dma_start(T3_d.ap()[NA:NA + 1, :], zrow[:])
                s4w = nc.sync.dma_start(T4_d.ap()[NA:NA + 1, :], zrow[:])
                wf3 = emit_wfull(nc, pools, W3_d.ap(), w3t_t[:], asf3_t[:], adf3_t[:], 48, 32, 2, "3")
                wf4 = emit_wfull(nc, pools, W4_d.ap(), w4t_t[:], asf4_t[:], adf4_t[:], 32, 10, 1, "4")
                rn2 = emit_rnorm(nc, pools, p2_t[:], 10)
                # all compact nodes are active: s = tanh(score1), no masking
                s_t = resp.tile([128, NCH2], F32, name="s_t")
                nc.scalar.activation(s_t[:], sc1_t[:], AF.Tanh)

                t3w = [[] for _ in range(GPC)]
                for c in range(NCH2):
                    emit_produce(ctx, 3, c, x3T[:, c * 128:(c + 1) * 128], wf3, T3_d,
                                 vv3[:], scale_col=s_t[:, c:c + 1],
                                 trow_writes=t3w[c // JCH2])
                bars3 = emit_graph_barriers(nc, constp, t3w, s3w, "t3")
                for c in range(NCH2):
                    rows = emit_attention(ctx, 3, c, Kc[c], goffs[c], idx_t, T3_d,
                                          vv3[:], b3_t[:],
                                          gather_dep=[bars3[c // JCH2], lib])
                    emit_transpose(ctx, rows, 32, x4T[:, c * 128:(c + 1) * 128])
                t4w = [[] for _ in range(GPC)]
                for c in range(NCH2):
                    emit_produce(ctx, 4, c, x4T[:, c * 128:(c + 1) * 128], wf4, T4_d,
                                 vv4[:], trow_writes=t4w[c // JCH2])
                bars4 = emit_graph_barriers(nc, constp, t4w, s4w, "t4")
                x5st = resp.tile([128, NCH2 * 10], F32, name="x5st")
                sc2 = resp.tile([128, NCH2], F32, name="sc2")
                for c in range(NCH2):
                    rows = emit_attention(ctx, 4, c, Kc[c], goffs[c], idx_t, T4_d,
                                          vv4[:], b4_t[:],
                                          gather_dep=[bars4[c // JCH2], lib])
                    nc.vector.tensor_copy(x5st[:, c * 10:(c + 1) * 10], rows[:])
                    emit_score(ctx, rows, 10, p2_t[:], rn2, sc2[:, c:c + 1])
                nc.sync.dma_start(x5_d.ap(), x5st[:])
                nc.sync.dma_start(sc2_d.ap(), sc2[:])

            emit_reps_loop(nc, tc, resp, reps, body)
    nc.compile()
    return nc


# ------------------------------------------------------------------- K3
def build_k3():
    """pool2 apply + per-graph mean + log_softmax on compact layout -> [GPC, 10]."""
    nc = bacc.Bacc("TRN2", target_bir_lowering=False, debug=False, num_devices=8)
    x5_d = nc.dram_tensor("x5", [128, NCH2 * 10], F32, kind="ExternalInput")
    sc2_d = nc.dram_tensor("score2", [128, NCH2], F32, kind="ExternalInput")
    nm2_d = nc.dram_tensor("nm2", [128, NCH2], F32, kind="ExternalInput")
    out_d = nc.dram_tensor("out", [GPC, 10], F32, kind="ExternalOutput")
    CPG = JCH2  # compact chunks per graph = 8
    with tile.TileContext(nc) as tc:
        with (
            tc.tile_pool(name="p", bufs=1) as pool,
            tc.tile_pool(name="psum", bufs=4, space="PSUM") as psump,
        ):
            x5 = pool.tile([128, NCH2 * 10], F32, name="x5")
            nc.sync.dma_start(x5[:], x5_d.ap())
            sc2 = pool.tile([128, NCH2], F32, name="sc2")
            nc.sync.dma_start(sc2[:], sc2_d.ap())
            nm2 = pool.tile([128, NCH2], F32, name="nm2")
            nc.sync.dma_start(nm2[:], nm2_d.ap())
            ident = pool.tile([128, 128], F32, name="ident")
            make_identity(nc, ident[:])
            s = pool.tile([128, NCH2], F32, name="s")
            nc.scalar.activation(s[:], sc2[:], AF.Tanh)
            nc.vector.tensor_tensor(s[:], s[:], nm2[:], op=ALU.mult)
            xn = pool.tile([128, NCH2 * 10], F32, name="xn")
            x5_3 = x5[:].rearrange("p (c f) -> p c f", f=10)
            s3 = s[:].unsqueeze(2).broadcast_to([128, NCH2, 10])
            nc.vector.tensor_tensor(xn[:].rearrange("p (c f) -> p c f", f=10), x5_3, s3, op=ALU.mult)
            ones = pool.tile([128, 1], F32, name="ones")
            nc.vector.memset(ones[:], 1.0)
            pg = psump.tile([10, GPC], F32, space="PSUM", name="pg")
            for g in range(GPC):
                for j in range(CPG):
                    c = g * CPG + j
                    nc.tensor.matmul(pg[:, g:g + 1], lhsT=xn[:, c * 10:(c + 1) * 10],
                                     rhs=ones[:], start=(j == 0), stop=(j == CPG - 1))
            pooled_t = pool.tile([10, GPC], F32, name="pooledt")
            nc.vector.tensor_scalar_mul(pooled_t[:], pg[:], 1.0 / (NPER // 4))
            ptr = psump.tile([GPC, 10], F32, space="PSUM", name="ptr")
            nc.tensor.transpose(out=ptr[:], in_=pooled_t[:], identity=ident[:10, :10])
            pooled = pool.tile([GPC, 10], F32, name="pooled")
            nc.vector.tensor_copy(pooled[:], ptr[:])
            m = pool.tile([GPC, 2], F32, name="m")
            nc.vector.tensor_reduce(m[:, 0:1], pooled[:], axis=AX.X, op=ALU.max)
            nc.vector.tensor_scalar_mul(m[:, 1:2], m[:, 0:1], -1.0)
            ex = pool.tile([GPC, 10], F32, name="ex")
            se = pool.tile([GPC, 1], F32, name="se")
            nc.scalar.activation(ex[:], pooled[:], AF.Exp, bias=m[:, 1:2], accum_out=se[:])
            lse = pool.tile([GPC, 1], F32, name="lse")
            nc.scalar.activation(lse[:], se[:], AF.Ln)
            o = pool.tile([GPC, 10], F32, name="o")
            nc.vector.tensor_scalar(o[:], pooled[:], scalar1=m[:, 1:2], scalar2=None,
                                    op0=ALU.add)
            nc.vector.tensor_scalar(o[:], o[:], scalar1=lse[:], scalar2=None,
                                    op0=ALU.subtract)
            nc.sync.dma_start(out_d.ap(), o[:])
    nc.compile()
    return nc


# ===== top-level driver =====
from concourse import bass_utils

_CACHE = {}


def _get_programs(Kw, reps=1):
    key = (tuple(int(k) for k in Kw), reps)
    if key not in _CACHE:
        _CACHE[key] = (build_k0(), build_k1(Kw, reps=reps), build_k2(Kw, reps=reps),
                       build_k3())
    return _CACHE[key]


def _run(nc, feeds):
    res = bass_utils.run_bass_kernel_spmd(nc, feeds, core_ids=list(range(NCORES)))
    return res.results


def make_feeds(inputs, st):
    pk = st["packs"]
    feeds0 = [dict(xT=st["percore"][c]["xT"]) for c in range(NCORES)]
    common1 = dict(gamma=np.asarray(inputs["gamma"], np.float32).reshape(128, 1),
                   beta=np.asarray(inputs["beta"], np.float32).reshape(128, 1),
                   W1=inputs["W1"], W1T=np.ascontiguousarray(inputs["W1"].T),
                   W2=inputs["W2"], W2T=np.ascontiguousarray(inputs["W2"].T),
                   asf1=pk["asf1"], adf1=pk["adf1"], asf2=pk["asf2"], adf2=pk["adf2"],
                   b1=inputs["b1"].reshape(1, -1), b2=inputs["b2"].reshape(1, -1),
                   p1=inputs["p1"].reshape(1, -1))
    common2 = dict(W3=inputs["W3"], W3T=np.ascontiguousarray(inputs["W3"].T),
                   W4=inputs["W4"], W4T=np.ascontiguousarray(inputs["W4"].T),
                   asf3=pk["asf3"], adf3=pk["adf3"], asf4=pk["asf4"], adf4=pk["adf4"],
                   b3=inputs["b3"].reshape(1, -1), b4=inputs["b4"].reshape(1, -1),
                   p2=inputs["p2"].reshape(1, -1))
    return feeds0, common1, common2


def kernel(**inputs):
    inputs = {k: np.asarray(v) for k, v in inputs.items()}
    st = prep_static(inputs)
    Kw = st["K"]
    nc0, nc1, nc2, nc3 = _get_programs(Kw)
    feeds0, common1, common2 = make_feeds(inputs, st)

    o0 = _run(nc0, feeds0)
    stats = np.sum([o["stats"] for o in o0], axis=0).astype(np.float32)

    feeds1 = [dict(xT=st["percore"][c]["xT"], idx=st["percore"][c]["idx16"],
                   stats=stats, **common1) for c in range(NCORES)]
    o1 = _run(nc1, feeds1)

    sc1 = [o["score1"] for o in o1]
    nm1 = host_topk(sc1, st["perms"], None, NPER // 2)

    feeds2 = [dict(x3T=o1[c]["x3T"], score1=sc1[c], nm1=nm1[c],
                   idx=st["percore"][c]["idx16"], **common2) for c in range(NCORES)]
    o2 = _run(nc2, feeds2)

    sc2 = [o["score2"] for o in o2]
    nm2 = host_topk(sc2, st["perms"], nm1, NPER // 4)

    feeds3 = [dict(x5=o2[c]["x5"], score2=sc2[c], nm2=nm2[c]) for c in range(NCORES)]
    o3 = _run(nc3, feeds3)
    out = np.concatenate([o["out"] for o in o3], axis=0).astype(np.float32)
    return out
